# revision 51
# baseline (speedup 1.0000x reference)
"""Self-contained Trainium2 (Bass/Tile) kernel for the AttentionGRUCell
problem: 8-core data-parallel over batch, fp8/bf16 matmuls (<2e-2 rel err).

kernel(**inputs) takes the FULL unsharded inputs and returns the FULL
[512, 1088] output ([alpha, h_new] per row), running the Bass program on
NeuronCores 0-7 via run_bass_kernel_spmd.

Design notes (fp8 DoubleRow rewrite):
- The attention main GEMM tanh(es).T @ Wa_bot, the e-accumulation
  (Va . tanh(g)), the context matmul (alpha-masks @ es) and the GRU
  kernel ct-part streams all run as fp8(e4m3) DoubleRow matmuls: 256-row
  contraction per pass, ~1.2x the bf16 stream rate per instruction and
  half the instruction count. Host-side scales (Wa_bot x64, Va x16,
  kernel-ct x64, ct x8 on device) keep the fp8 encodings in the normal
  range; the scales are undone for free in the STT that folds the qk
  add / gate-bias adds (scalar=1/scale) and in the exp (scale=1/16).
- Error-critical paths stay bf16: qk (Wa_top), h@rk, inputs@kernel_x,
  (r*h)@rk_hh. hard_sigmoid's 0.2 slope and the smallness of the
  ct-part preactivation (std ~0.14) keep the fp8 error ~<0.5% of h_new.
- e-acc pairs lag half a superblock so the PE never waits on the
  STT+tanh chain; softmax/ct shift one superblock later (ct lag-1 kept).
- tesT/gT tanh are emitted in j-pairs ([128,1024] per ACT) to halve the
  ~350ns per-instruction ACT overhead.
- Startup: hT first, then esT0/wab chunk-interleaved so the first main
  matmuls start as soon as the first 256-row pair of tanh(es) lands.
- Tail: ct rows 0..55 transpose during g7; only the last 8 rows +
  copy-casts + DR streams remain after the loop, keeping the PE dense
  enough that the HAM clock stays warm.
"""
import sys

for _p in ("/opt/trn_rl_repo",):
    if _p not in sys.path:
        sys.path.insert(0, _p)

import numpy as np
import concourse.bass as bass
import concourse.mybir as mybir
import concourse.tile as tile
import bass_rust
from concourse.alu_op_type import AluOpType
from concourse.masks import make_identity
from concourse.vector_clock import ScopedClock

F32 = mybir.dt.float32
BF = mybir.dt.bfloat16
F8 = mybir.dt.float8e4
DR = mybir.MatmulPerfMode.DoubleRow
AF = mybir.ActivationFunctionType
AX = mybir.AxisListType

# host-side fp8 range scales, undone on device for free (STT scalars / exp
# scale).
WA_SCALE = 64.0   # Wa_bot
VA_SCALE = 16.0   # Va
KC_SCALE = 64.0   # kernel ct-part rows
CT_SCALE = 8.0    # ct (applied on device via srec8)

Bc, T, XD, ED, U = 64, 64, 512, 1024, 1024
NSBLK = 8
N_CORES = 8
B_FULL = 512


# ---------------------------------------------------------------------------
# Workarounds for this walrus build: instructions may carry at most one sem
# wait ("Too many sync wait commands"), including the Tile kernel-tail drain.
# ---------------------------------------------------------------------------

def _patched_drain_and_barrier(self, tick_clock, wait_clock):
    nc = self.nc
    probe = nc.sync.nop(nofuse=True)
    wait_clock.add_sem_waits(probe.ins, ScopedClock({None: tick_clock.global_clock}))
    si = probe.ins.sync_info
    waits = list(si.on_wait) if si is not None else []
    probe.ins.sync_info = bass_rust.SyncInfo(on_wait=waits[:1], on_update=[])
    for w in waits[1:]:
        n2 = nc.sync.nop(nofuse=True)
        n2.ins.sync_info = bass_rust.SyncInfo(on_wait=[w], on_update=[])
    nc.sync.drain()
    nc.all_engine_barrier()
    assert self.sems is not None
    popped = nc._tile_sem_poison_stack.pop()
    assert popped is self._sem_poison
    nc.clear_and_free_semaphores(list(self.sems.allocated().values()))
    nc.all_engine_barrier()


tile.TileContext._drain_and_barrier = _patched_drain_and_barrier

_fix_ctr = [0]


def fix_multi_waits(nc, max_waits=1):
    """Hoist extra sem waits onto same-engine InstNoOps placed immediately
    before the instruction -- engines execute in order, so semantics are
    identical."""
    for f in nc.m.functions:
        for blk in f.blocks:
            insts = blk.instructions
            if not any(
                i.sync_info is not None and len(i.sync_info.on_wait) > max_waits
                for i in insts
            ):
                continue
            out = []
            for inst in insts:
                si = inst.sync_info
                if si is not None and len(si.on_wait) > max_waits:
                    waits = list(si.on_wait)
                    for w in waits[:-max_waits]:
                        _fix_ctr[0] += 1
                        nop = mybir.InstNoOp(
                            name=f"waitfix-{_fix_ctr[0]}",
                            ins=[],
                            outs=[],
                            engine=inst.engine,
                        )
                        nop.sync_info = bass_rust.SyncInfo(on_wait=[w], on_update=[])
                        out.append(nop)
                    inst.sync_info = bass_rust.SyncInfo(
                        on_wait=waits[-max_waits:], on_update=list(si.on_update)
                    )
                out.append(inst)
            blk.instructions = out


# ---------------------------------------------------------------------------
# Kernel program
# ---------------------------------------------------------------------------

def build_nc():
    nc = bass.Bass("TRN2", target_bir_lowering=False, debug=False)

    inT_d = nc.dram_tensor("inT", [128, 4, Bc], BF, kind="ExternalInput")
    h_d = nc.dram_tensor("h", [Bc, U], F32, kind="ExternalInput")
    hT_d = nc.dram_tensor("hT", [128, 8, Bc], BF, kind="ExternalInput")
    es_d = nc.dram_tensor("es", [8, 128, 4, ED], F8, kind="ExternalInput")
    esT_d = nc.dram_tensor("esT", [8, 128, 8, 512], F8, kind="ExternalInput")
    kernx_d = nc.dram_tensor("kernx", [XD, 3 * U], BF, kind="ExternalInput")
    kernc_d = nc.dram_tensor("kernc", [ED, 3 * U], F8, kind="ExternalInput")
    rk_d = nc.dram_tensor("rk", [U, 3 * U], BF, kind="ExternalInput")
    bias_d = nc.dram_tensor("bias", [3 * U], F32, kind="ExternalInput")
    wa_d = nc.dram_tensor("wab", [ED, U], F8, kind="ExternalInput")
    wat_d = nc.dram_tensor("wat", [8, 128, 8, 128], BF, kind="ExternalInput")
    va_d = nc.dram_tensor("va", [128, 2, 16], F8, kind="ExternalInput")
    out_d = nc.dram_tensor("out", [Bc, T + U], F32, kind="ExternalOutput")

    with tile.TileContext(nc) as tc:
        with (
            tc.tile_pool(name="singles", bufs=1) as sg,
            tc.tile_pool(name="esT", bufs=2) as esT_pool,
            tc.tile_pool(name="esn", bufs=3) as esn_pool,
            tc.tile_pool(name="tesT", bufs=2) as tesT_pool,
            tc.tile_pool(name="gT", bufs=1) as gT_pool,
            tc.tile_pool(name="gst", bufs=4) as gst_pool,
            tc.tile_pool(name="wk", bufs=2) as wk_pool,
            tc.tile_pool(name="kx", bufs=2) as kx_pool,
            tc.tile_pool(name="kc", bufs=3) as kc_pool,
            tc.tile_pool(name="smalls", bufs=4) as sm_pool,
            tc.tile_pool(name="ps_v", bufs=3, space="PSUM") as ps_v,
            tc.tile_pool(name="ps_tr", bufs=1, space="PSUM") as ps_tr,
            tc.tile_pool(name="ps_acc", bufs=2, space="PSUM") as ps_acc,
            tc.tile_pool(name="ps_e", bufs=1, space="PSUM") as ps_e,
            tc.tile_pool(name="ps_ct", bufs=1, space="PSUM") as ps_ct,
        ):
            # ---- startup DMAs, spread over engine queues: each dma_start
            # costs ~640ns of enqueue time on its issuing engine, so the
            # critical esT0/wab chunks get the sync queue to themselves ----
            hT_sb = sg.tile([128, 8, Bc], BF)
            nc.gpsimd.dma_start(out=hT_sb[:], in_=hT_d[:])

            # esT0 / wab chunk-interleaved: the first DR main matmul only
            # needs pair 0 of both.
            esT_cur = esT_pool.tile([128, 8, 512], F8, tag="esT", name="esT0")
            wab_sb = sg.tile([128, 8, U], F8)
            # first column-half of wab only: mains u0..u3 need wab cols
            # 0:512, so the critical startup prefix is 1MB, not 1.5MB
            for q in range(4):
                nc.sync.dma_start(
                    out=esT_cur[:, 2 * q:2 * q + 2, :],
                    in_=esT_d.ap()[0, :, 2 * q:2 * q + 2, :],
                )
                nc.sync.dma_start(
                    out=wab_sb[:, 2 * q:2 * q + 2, 0:512],
                    in_=wa_d.ap()[256 * q:256 * (q + 1), 0:512].rearrange(
                        "(i p) c -> p i c", i=2
                    ),
                )
            for q in range(4):
                nc.sync.dma_start(
                    out=wab_sb[:, 2 * q:2 * q + 2, 512:1024],
                    in_=wa_d.ap()[256 * q:256 * (q + 1), 512:1024].rearrange(
                        "(i p) c -> p i c", i=2
                    ),
                )

            va_sb = sg.tile([128, 2, 16], F8)
            nc.gpsimd.dma_start(out=va_sb[:], in_=va_d[:])
            # qk weight blocks: two enqueues on the sync queue AFTER the
            # critical esT0/wab chunks (priority = sync enqueue order)
            wau_all = sg.tile([128, 8, 8, 128], BF)
            for uh in range(2):
                nc.sync.dma_start(
                    out=wau_all[:, 4 * uh:4 * (uh + 1), :, :],
                    in_=wat_d.ap()[4 * uh:4 * (uh + 1)].rearrange(
                        "u p d c -> p u d c"
                    ),
                )
            h_sb = sg.tile([Bc, U], F32)
            nc.sync.dma_start(out=h_sb[:], in_=h_d[:])
            inT_sb = sg.tile([128, 4, Bc], BF)
            nc.sync.dma_start(out=inT_sb[:], in_=inT_d[:])

            def load_esT(g):
                t_ = esT_pool.tile([128, 8, 512], F8, tag="esT", name=f"esT{g}")
                nc.sync.dma_start(out=t_[:], in_=esT_d.ap()[g])
                return t_

            esn_tiles = {}

            def load_esn(g):
                t_ = esn_pool.tile([128, 4, ED], F8, tag="esn", name=f"esn{g}")
                nc.sync.dma_start(out=t_[:], in_=es_d.ap()[g])
                esn_tiles[g] = t_

            # warm the ACT table while the first DMAs are in flight
            scr1 = sg.tile([1, 1], F32)
            nc.vector.memset(scr1[:], 0.0)
            nc.scalar.activation(out=scr1[:], in_=scr1[:], func=AF.Tanh)

            bias_b = sg.tile([Bc, 3 * U], F32)
            # one partition-broadcast enqueue for the whole bias
            _bsrc = bias_d.ap()
            nc.gpsimd.dma_start(
                out=bias_b[:],
                in_=bass.AP(
                    tensor=_bsrc.tensor,
                    offset=_bsrc.offset,
                    ap=[[0, Bc], [1, 3 * U]],
                ),
            )

            esT_tiles = {0: esT_cur}

            thT = sg.tile([128, 8, Bc], BF)

            ident = sg.tile([16, 16], F32)
            make_identity(nc, ident[:])
            identb = sg.tile([Bc, Bc], BF)
            nc.vector.memset(identb[:], 0.0)
            nc.gpsimd.affine_select(
                out=identb[:],
                in_=identb[:],
                compare_op=mybir.AluOpType.not_equal,
                fill=1.0,
                base=0,
                pattern=[[-1, Bc]],
                channel_multiplier=1,
            )

            masks = sg.tile([128, 4, 8], F32)
            nc.vector.memset(masks[:], 0.0)
            for rr in range(4):
                nc.vector.memset(masks[0:64, rr, 2 * rr:2 * rr + 1], 1.0)
                nc.vector.memset(masks[64:128, rr, 2 * rr + 1:2 * rr + 2], 1.0)

            half_sb = sg.tile([Bc, 1], F32)
            nc.vector.memset(half_sb[:], 0.5)

            qkT = sg.tile([128, 8, Bc], BF)
            ctT = sg.tile([128, 8, Bc], F8)
            gh_sb = sg.tile([Bc, 2 * U], BF)
            gx0_sb = sg.tile([Bc, 3 * U], BF)
            ct_sb = sg.tile([Bc, ED], BF)       # rows 0..55 used (g0..g6)
            ct_sb7 = sg.tile([8, ED], BF)       # g7's rows (base-0 partitions)

            def qk_block_u(u):
                # qkT[u-block] = (Wa_top.T @ tanh(h).T) via small direct matmuls
                pq = ps_tr.tile([128, Bc], F32, tag="tr", name=f"pq{u}")
                for d in range(8):
                    nc.tensor.matmul(
                        pq[:],
                        wau_all[:, u, d, :],
                        thT[:, d, :],
                        start=(d == 0),
                        stop=(d == 7),
                    )
                nc.vector.tensor_copy(qkT[:, u, :], pq[:])

            # --- GRU weight-stream pieces: one batched enqueue per 512-col
            # slab, consumed d-granular by matmuls in the u-loop ---
            rkz_tiles = {}

            def load_rkz(n):
                # two half-slab enqueues so d=0 matmuls don't wait the full MB
                t_ = wk_pool.tile([128, 8, 512], BF, tag="wk", name=f"rkz{n}")
                for dh in range(2):
                    nc.sync.dma_start(
                        out=t_[:, 4 * dh:4 * (dh + 1), :],
                        in_=rk_d.ap()[
                            512 * dh:512 * (dh + 1), 512 * n:512 * (n + 1)
                        ].rearrange("(d p) c -> p d c", d=4),
                    )
                rkz_tiles[n] = t_

            def gh_step(n, d, pg):
                nc.tensor.matmul(
                    pg[:], hT_sb[:, d, :], rkz_tiles[n][:, d, :],
                    start=(d == 0), stop=(d == 7),
                )

            def gh_fin(n, pg):
                nc.vector.scalar_tensor_tensor(
                    out=gh_sb[:, 512 * n:512 * (n + 1)],
                    in0=pg[:],
                    scalar=1.0,
                    in1=bias_b[:, 512 * n:512 * (n + 1)],
                    op0=AluOpType.mult,
                    op1=AluOpType.add,
                )

            kx_tiles = {}

            def load_kx(n):
                t_ = kx_pool.tile([128, 4, 512], BF, tag="kx", name=f"kx{n}")
                for dh in range(2):
                    nc.sync.dma_start(
                        out=t_[:, 2 * dh:2 * (dh + 1), :],
                        in_=kernx_d.ap()[
                            256 * dh:256 * (dh + 1), 512 * n:512 * (n + 1)
                        ].rearrange("(d p) c -> p d c", d=2),
                    )
                kx_tiles[n] = t_

            def gx0_step(n, d, pa):
                nc.tensor.matmul(
                    pa[:], inT_sb[:, d, :], kx_tiles[n][:, d, :],
                    start=(d == 0), stop=(d == 3),
                )

            # --- pipelined e-acc -> alpha -> c_t pieces (lagged) ---
            pe_tiles = {}

            def eacc_pair(g, q, gT_g):
                # e[g] += (va_q).T @ gT[pair q]  -- fp8 DoubleRow, K=256
                if q == 0:
                    pe_tiles[g] = ps_e.tile([1, 512], F32, tag="e", name=f"pe{g}")
                nc.tensor.matmul(
                    pe_tiles[g][:],
                    va_sb[:, :, q:q + 1],
                    gT_g[:, 2 * q:2 * q + 2, :],
                    start=(q == 0),
                    stop=(q == 3),
                    perf_mode=DR,
                )

            def softmax(g):
                # alpha stays UNNORMALIZED; ct is scaled by CT_SCALE/sum via
                # srec8. exp undoes the VA_SCALE on e for free.
                e_sb = sm_pool.tile([1, 512], F32, tag="esb", name=f"esb{g}", bufs=1)
                nc.vector.tensor_copy(e_sb[:], pe_tiles[g][:])
                alpha = sm_pool.tile([8, T], F32, tag="al", name=f"al{g}")
                # scalar queue: hardware-dynamic and nearly empty, so this
                # latency-critical reshape never starves behind the bulk
                # weight slabs (gpsimd's software queue stalls ~7us at g6/g7)
                nc.scalar.dma_start(
                    out=alpha[:],
                    in_=e_sb[0:1, :].rearrange("p (b t) -> p b t", b=8),
                )
                ssum = sm_pool.tile([8, 1], F32, tag="ssum", name=f"ss{g}")
                nc.scalar.activation(
                    out=alpha[:], in_=alpha[:], func=AF.Exp,
                    scale=1.0 / VA_SCALE, accum_out=ssum[:],
                )
                srec = sm_pool.tile([8, 1], F32, tag="srec", name=f"sr{g}")
                nc.vector.reciprocal(srec[:], ssum[:])
                srec8 = sm_pool.tile([8, 1], F32, tag="srec8", name=f"sr8{g}")
                nc.vector.tensor_scalar_mul(srec8[:], srec[:], CT_SCALE)
                alpha_n = sm_pool.tile([8, T], F32, tag="aln", name=f"aln{g}")
                nc.vector.tensor_scalar_mul(alpha_n[:], alpha[:], srec[:])
                nc.gpsimd.dma_start(
                    out=out_d.ap()[8 * g:8 * (g + 1), 0:T], in_=alpha_n[:]
                )
                return alpha, srec8

            def ct_head(g, alpha):
                pat = ps_ct.tile([T, 8], F32, tag="ct", name=f"pat{g}")
                nc.tensor.transpose(pat[:], alpha[:], ident[:8, :8])
                alpT2 = sm_pool.tile([128, 8], F32, tag="alT2", name=f"aT2{g}")
                nc.vector.tensor_copy(alpT2[0:64, :], pat[:])
                nc.vector.tensor_copy(alpT2[64:128, :], pat[:])
                a2s = []
                for q in range(2):
                    a2 = sm_pool.tile([128, 2, 16], F8, tag="A", name=f"A{g}_{q}")
                    for i in range(2):
                        nc.gpsimd.tensor_mul(
                            a2[:, i, 0:8], alpT2[:], masks[:, 2 * q + i, :]
                        )
                    a2s.append(a2)
                return a2s

            def ct_tail(g, esn_g, a2s, srec8_g):
                # fp8 DR: out[8b, 512e] accumulating rr-pairs; g7's rows land
                # in the base-0 ct_sb7 tile directly, earlier rows go through
                # a staging tile + DMA (DVE writes need 32-aligned partition
                # bases).
                if g == 7:
                    stage = ct_sb7
                else:
                    stage = sm_pool.tile(
                        [8, ED], BF, tag="ctst", name=f"cts{g}", bufs=1
                    )
                for n in range(2):
                    pct = ps_ct.tile([8, 512], F32, tag="ct", name=f"pct{g}_{n}")
                    for q in range(2):
                        nc.tensor.matmul(
                            pct[:],
                            a2s[q][:, :, 0:8],
                            esn_g[:, 2 * q:2 * q + 2, 512 * n:512 * (n + 1)],
                            start=(q == 0),
                            stop=(q == 1),
                            perf_mode=DR,
                        )
                    nc.vector.tensor_scalar_mul(
                        stage[:, 512 * n:512 * (n + 1)], pct[:], srec8_g[:]
                    )
                if g != 7:
                    nc.gpsimd.dma_start(
                        out=ct_sb[8 * g:8 * (g + 1), :], in_=stage[:]
                    )

            # GRU ct-part weight preloads (fp8): one enqueue per gate part,
            # shaped [128, 4dcp, 2i, 1024] so DR pair slices come out 3D.
            kc_tiles = {}

            def preload_tail_part(part):
                wp = kc_pool.tile(
                    [128, 4, 2, U], F8, tag="kc", name=f"kc{part}"
                )
                nc.sync.dma_start(
                    out=wp[:],
                    in_=kernc_d.ap()[:, U * part:U * (part + 1)].rearrange(
                        "(q i p) c -> p q i c", q=4, i=2
                    ),
                )
                kc_tiles[part] = wp

            def emit_tesT_tanh(dst, src):
                for q in range(4):
                    nc.scalar.activation(
                        out=dst[:, 2 * q:2 * q + 2, :],
                        in_=src[:, 2 * q:2 * q + 2, :],
                        func=AF.Tanh,
                    )

            rkh_all = sg.tile([128, 8, U], BF)

            _dummy_ctr = [0]

            def dummy_mm(k):
                # dep-free 64-col transposes: ~150ns each of PE activity to
                # keep the HAM clock warm across short serial waits
                for _ in range(k):
                    _dummy_ctr[0] += 1
                    pdm = ps_tr.tile(
                        [Bc, Bc], BF, tag="tr", name=f"dum{_dummy_ctr[0]}"
                    )
                    nc.tensor.transpose(pdm[:], identb[:], identb[:])

            # ---- attention superblock loop ----
            load_rkz(0)
            tesT_cur = tesT_pool.tile([128, 8, 512], F8, tag="tesT", name="tes0")
            emit_tesT_tanh(tesT_cur, esT_cur)
            nc.scalar.activation(out=thT[:], in_=hT_sb[:], func=AF.Tanh)

            gT_tiles = {}
            alpha_info = {}   # g -> (alpha, srec8)
            a2s_prev = None

            for g in range(NSBLK):
                gh_n = g - 1 if 1 <= g <= 4 else None
                # hh-column chunks (n=4,5) move to the post-loop tail as
                # g7-independent PE filler
                gx_n = g - 2 if 2 <= g <= 5 else None
                pg = (
                    ps_acc.tile([Bc, 512], F32, tag="acc", name=f"pg{gh_n}")
                    if gh_n is not None else None
                )
                pa = (
                    ps_acc.tile([Bc, 512], F32, tag="acc", name=f"gx0_{gx_n}")
                    if gx_n is not None else None
                )

                tesT = tesT_cur
                ilv = 1 <= g < NSBLK - 1   # interleave next tesT tanh (g>=1)
                if ilv:
                    tesT_nxt = tesT_pool.tile(
                        [128, 8, 512], F8, tag="tesT", name=f"tes{g+1}"
                    )
                gT = gT_pool.tile([128, 8, 512], F8, tag="gT", name=f"gt{g}")
                gT_tiles[g] = gT
                gstage = gst_pool.tile(
                    [128, 2, 512], BF, tag="gst", name=f"gs{g}"
                )
                for u in range(8):
                    pv = ps_v.tile([128, 512], F32, tag="v", name=f"pv{g}_{u}")
                    for j2 in range(4):
                        # fp8 DoubleRow: contracts 256 rows per pass
                        nc.tensor.matmul(
                            pv[:],
                            wab_sb[:, 2 * j2:2 * j2 + 2, 128 * u:128 * (u + 1)],
                            tesT[:, 2 * j2:2 * j2 + 2, :],
                            start=(j2 == 0),
                            stop=(j2 == 3),
                            perf_mode=DR,
                        )
                    if g == 0:
                        # must precede the STT below, which reads qkT[:, u, :]
                        qk_block_u(u)
                    qk_slice = qkT[:, u, 8 * g:8 * g + 8]
                    qk_bc = bass.AP(
                        tensor=qk_slice.tensor,
                        offset=qk_slice.offset,
                        ap=[
                            list(qk_slice.ap[0]),
                            list(qk_slice.ap[1]),
                            [0, T],
                        ],
                    )
                    nc.vector.scalar_tensor_tensor(
                        out=gstage[:, u % 2, :],
                        in0=pv[:],
                        scalar=1.0 / WA_SCALE,
                        in1=qk_bc,
                        op0=AluOpType.mult,
                        op1=AluOpType.add,
                    )
                    if u % 2 == 1:
                        # batched pair tanh -> fp8 gT
                        nc.scalar.activation(
                            out=gT[:, u - 1:u + 1, :],
                            in_=gstage[:],
                            func=AF.Tanh,
                        )
                        gstage = gst_pool.tile(
                            [128, 2, 512], BF, tag="gst", name=f"gs{g}_{u}"
                        )
                        if ilv:
                            # next superblock's tesT tanh pair
                            qq = (u - 1) // 2
                            nc.scalar.activation(
                                out=tesT_nxt[:, 2 * qq:2 * qq + 2, :],
                                in_=esT_tiles[g + 1][:, 2 * qq:2 * qq + 2, :],
                                func=AF.Tanh,
                            )
                    # lagged e-acc pairs: q0/q1 of g at u=5/7, q2/q3 of g-1
                    # at u=0/2. softmax starts at u=2 and ct_head waits to
                    # u=6 so the alpha-reshape DMA's software-queue latency
                    # jitter (multi-us) never stalls the PE at pat.
                    if u == 5:
                        eacc_pair(g, 0, gT)
                    elif u == 7:
                        eacc_pair(g, 1, gT)
                    elif u == 0 and g >= 1:
                        eacc_pair(g - 1, 2, gT_tiles[g - 1])
                    elif u == 2 and g >= 1:
                        eacc_pair(g - 1, 3, gT_tiles[g - 1])
                    if u == 2 and g >= 1:
                        alpha_info[g - 1] = softmax(g - 1)
                    if u == 5 and g >= 1:
                        a2s_prev = ct_head(g - 1, alpha_info[g - 1][0])
                    if u == 7 and g >= 1:
                        ct_tail(
                            g - 1, esn_tiles[g - 1], a2s_prev,
                            alpha_info[g - 1][1],
                        )
                    if gh_n is not None:
                        gh_step(gh_n, u, pg)
                    if gx_n is not None and u < 4:
                        gx0_step(gx_n, u, pa)

                if gh_n is not None:
                    gh_fin(gh_n, pg)
                if gx_n is not None:
                    # fold gh (z/r cols) into the gx0 store so the GRU tail
                    # does a single add per gate column
                    nc.vector.scalar_tensor_tensor(
                        out=gx0_sb[:, 512 * gx_n:512 * (gx_n + 1)],
                        in0=pa[:],
                        scalar=1.0,
                        in1=gh_sb[:, 512 * gx_n:512 * (gx_n + 1)],
                        op0=AluOpType.mult,
                        op1=AluOpType.add,
                    )

                if ilv:
                    tesT_cur = tesT_nxt
                elif g == 0:
                    # g0: batch-tanh tesT(1) after the u-loop (esT1 lands late)
                    esT_tiles[1] = load_esT(1)
                    tesT_cur = tesT_pool.tile(
                        [128, 8, 512], F8, tag="tesT", name="tes1"
                    )
                    emit_tesT_tanh(tesT_cur, esT_tiles[1])
                if g + 2 < NSBLK:
                    esT_tiles[g + 2] = load_esT(g + 2)

                # spread remaining weight streams across the attention phase
                if g == 0:
                    load_esn(0)
                    load_esn(1)
                    load_esn(2)
                elif g + 2 < NSBLK:
                    load_esn(g + 2)
                if 1 <= g <= 3:
                    load_rkz(g)      # h@rk slab for gh at g+1
                if 1 <= g <= 6:
                    load_kx(g - 1)   # inputs@kernel slab for gx0 at g+1
                if g == 5:
                    preload_tail_part(0)
                if g == 6:
                    preload_tail_part(1)
                    nc.sync.dma_start(
                        out=rkh_all[:],
                        in_=rk_d.ap()[:, 2 * U:].rearrange(
                            "(d p) c -> p d c", d=8
                        ),
                    )
                if g == 7:
                    preload_tail_part(2)

            # ---- post-loop: finish g7's e/alpha/ct; keep the PE dense with
            # g7-independent filler so the HAM clock never drops ----
            eacc_pair(7, 2, gT_tiles[7])
            eacc_pair(7, 3, gT_tiles[7])
            alpha_info[7] = softmax(7)
            dummy_mm(4)

            # real filler: inputs-part gx0 for the hh columns (n=4,5)
            for n45 in (4, 5):
                pa45 = ps_acc.tile([Bc, 512], F32, tag="acc", name=f"gx0_{n45}")
                for d in range(4):
                    gx0_step(n45, d, pa45)
                nc.vector.scalar_tensor_tensor(
                    out=gx0_sb[:, 512 * n45:512 * (n45 + 1)],
                    in0=pa45[:],
                    scalar=1.0,
                    in1=bias_b[:, 512 * n45:512 * (n45 + 1)],
                    op0=AluOpType.mult,
                    op1=AluOpType.add,
                )

            # pre-transpose ct rows 0..55 (g0..g6) into fp8 ctT -- fills the
            # PE while g7's softmax chain runs on gpsimd/ACT/DVE
            for j in range(8):
                pool, tag = (ps_tr, "tr") if j % 2 == 0 else (ps_ct, "ct")
                pt = pool.tile([128, 56], BF, tag=tag, name=f"tpc_{j}")
                nc.tensor.transpose(
                    pt[:], ct_sb[0:56, 128 * j:128 * (j + 1)],
                    identb[0:56, 0:56],
                )
                nc.scalar.copy(ctT[:, j, 0:56], pt[:])
            dummy_mm(4)
            a2s_prev = ct_head(7, alpha_info[7][0])
            dummy_mm(3)
            ct_tail(7, esn_tiles[7], a2s_prev, alpha_info[7][1])
            dummy_mm(4)

            # ---- GRU tail ----
            z_sb = sg.tile([Bc, U], F32)
            r_sb = sg.tile([Bc, U], F32)
            rh_bf = sg.tile([Bc, U], BF)
            rhT = sg.tile([128, 8, Bc], BF)
            zh_sb = sg.tile([Bc, U], F32)
            omz_sb = sg.tile([Bc, U], F32)
            hh_sb = r_sb  # r is dead once rh_bf is formed

            # six gate accumulators in the (now idle) attention psum banks
            gx = [
                ps_v.tile([Bc, 512], F32, tag="v", name="gxa"),
                ps_v.tile([Bc, 512], F32, tag="v", name="gxb"),
                ps_v.tile([Bc, 512], F32, tag="v", name="gxc"),
                ps_e.tile([Bc, 512], F32, tag="e", name="gxd"),
                ps_acc.tile([Bc, 512], F32, tag="acc", name="gxe"),
                ps_acc.tile([Bc, 512], F32, tag="acc", name="gxf"),
            ]

            # last 8 ct rows: transpose + copy-cast
            for j in range(8):
                pool, tag = (ps_tr, "tr") if j % 2 == 0 else (ps_ct, "ct")
                pt = pool.tile([128, 8], BF, tag=tag, name=f"tp8_{j}")
                nc.tensor.transpose(
                    pt[:], ct_sb7[:, 128 * j:128 * (j + 1)], identb[0:8, 0:8]
                )
                nc.scalar.copy(ctT[:, j, 56:64], pt[:])
            dummy_mm(3)

            # z/r ct-part: fp8 DR pairs (scale 1/(KC*CT) folded into the
            # adds). r gates (part 1) stream FIRST: their psums gate the
            # rhT transposes, so finishing them early overlaps the r-gate
            # chain with the z and hh-ct streams.
            for part in (1, 0):
                for dcp in range(4):
                    for half in range(2):
                        n = 2 * part + half
                        nc.tensor.matmul(
                            gx[n][:],
                            ctT[:, 2 * dcp:2 * dcp + 2, :],
                            kc_tiles[part][:, dcp, :, 512 * half:512 * (half + 1)],
                            start=(dcp == 0),
                            stop=(dcp == 3),
                            perf_mode=DR,
                        )

            GATE_RS = 1.0 / (KC_SCALE * CT_SCALE)

            def add_inplace(pa_, n, src_sb, scalar):
                nc.vector.scalar_tensor_tensor(
                    out=pa_[:],
                    in0=pa_[:],
                    scalar=scalar,
                    in1=src_sb[:, 512 * n:512 * (n + 1)],
                    op0=AluOpType.mult,
                    op1=AluOpType.add,
                )

            # hh ct-part stream first (keeps PE busy while the z/r gate chain
            # runs on ACT/DVE); rh contributions are pre-scaled x512 so the
            # shared psum stays on one scale
            for dcp in range(4):
                for half in range(2):
                    nc.tensor.matmul(
                        gx[4 + half][:],
                        ctT[:, 2 * dcp:2 * dcp + 2, :],
                        kc_tiles[2][:, dcp, :, 512 * half:512 * (half + 1)],
                        start=(dcp == 0),
                        stop=False,
                        perf_mode=DR,
                    )

            # gates: hard_sigmoid(gx + gx0 + gh) = min(relu(0.2x+0.5), 1).
            # r gates FIRST -- only r gates the PE-critical rhT transposes;
            # z feeds nothing until the final combine.
            dummy_mm(6)
            for n in (2, 3, 0, 1):
                dst = z_sb if n < 2 else r_sb
                o = 512 * (n % 2)
                sl = slice(o, o + 512)
                if n == 2:
                    # r-half0 gates the FIRST rhT transposes: run its chain
                    # in 256-col strips so ACT/DVE pipeline and transpose j0
                    # starts ~1.3us earlier (keeps the HAM clock warm)
                    for s2 in range(2):
                        ps = slice(256 * s2, 256 * (s2 + 1))
                        ss = slice(o + 256 * s2, o + 256 * (s2 + 1))
                        nc.vector.scalar_tensor_tensor(
                            out=gx[n][:, ps], in0=gx[n][:, ps],
                            scalar=GATE_RS,
                            in1=gx0_sb[:, 512 * n + 256 * s2:512 * n + 256 * (s2 + 1)],
                            op0=AluOpType.mult, op1=AluOpType.add,
                        )
                        nc.scalar.activation(
                            out=dst[:, ss], in_=gx[n][:, ps],
                            func=AF.Relu, bias=half_sb[:], scale=0.2,
                        )
                        nc.vector.tensor_scalar_min(dst[:, ss], dst[:, ss], 1.0)
                        nc.vector.scalar_tensor_tensor(
                            out=rh_bf[:, ss], in0=dst[:, ss],
                            scalar=1.0 / GATE_RS, in1=h_sb[:, ss],
                            op0=AluOpType.mult, op1=AluOpType.mult,
                        )
                        for j in range(2 * s2, 2 * s2 + 2):
                            pool, tag = (ps_tr, "tr") if j % 2 == 0 else (ps_ct, "ct")
                            pt = pool.tile([128, Bc], BF, tag=tag, name=f"tprh_{j}")
                            nc.tensor.transpose(
                                pt[:], rh_bf[:, 128 * j:128 * (j + 1)], identb[:]
                            )
                            nc.scalar.copy(rhT[:, j, :], pt[:])
                    continue
                add_inplace(gx[n], n, gx0_sb, GATE_RS)
                nc.scalar.activation(
                    out=dst[:, sl], in_=gx[n][:],
                    func=AF.Relu, bias=half_sb[:], scale=0.2,
                )
                nc.vector.tensor_scalar_min(dst[:, sl], dst[:, sl], 1.0)
                if n < 2:
                    # precompute z*h and (1-z) off the critical path
                    nc.vector.tensor_mul(zh_sb[:, sl], dst[:, sl], h_sb[:, sl])
                    nc.vector.tensor_scalar(
                        out=omz_sb[:, sl], in0=dst[:, sl],
                        scalar1=-1.0, scalar2=1.0,
                        op0=AluOpType.mult, op1=AluOpType.add,
                    )
                else:
                    # rh = (r * 512) * h so the hh psum (ct-part x512) stays
                    # single-scale
                    nc.vector.scalar_tensor_tensor(
                        out=rh_bf[:, sl],
                        in0=dst[:, sl],
                        scalar=1.0 / GATE_RS,
                        in1=h_sb[:, sl],
                        op0=AluOpType.mult,
                        op1=AluOpType.mult,
                    )
                    for j in range(4 * (n - 2), 4 * (n - 1)):
                        pool, tag = (ps_tr, "tr") if j % 2 == 0 else (ps_ct, "ct")
                        pt = pool.tile([128, Bc], BF, tag=tag, name=f"tprh_{j}")
                        nc.tensor.transpose(
                            pt[:], rh_bf[:, 128 * j:128 * (j + 1)], identb[:]
                        )
                        nc.scalar.copy(rhT[:, j, :], pt[:])

            # (r*h) @ rk_hh stream, bank-major so the first hh half finishes
            # early and its scalar/vector chain overlaps the second bank
            for n2 in range(2):
                for d in range(8):
                    nc.tensor.matmul(
                        gx[4 + n2][:],
                        rhT[:, d, :],
                        rkh_all[:, d, 512 * n2:512 * (n2 + 1)],
                        start=False,
                        stop=(d == 7),
                    )

            # hh = tanh(...); h_new = z*h + (1-z)*hh  (zh/omz precomputed)
            t1 = sg.tile([Bc, U], F32)
            for n2 in range(2):
                o = 512 * n2
                sl = slice(o, o + 512)
                pa = gx[4 + n2]
                add_inplace(pa, 4 + n2, gx0_sb, GATE_RS)
                nc.scalar.activation(out=hh_sb[:, sl], in_=pa[:], func=AF.Tanh)
                nc.vector.tensor_mul(t1[:, sl], omz_sb[:, sl], hh_sb[:, sl])
                nc.vector.tensor_add(t1[:, sl], t1[:, sl], zh_sb[:, sl])
                nc.sync.dma_start(out=out_d.ap()[:, T + o:T + o + 512], in_=t1[:, sl])

    return nc


_built = [None]


def _get_nc():
    if _built[0] is None:
        nc = build_nc()
        fix_multi_waits(nc)
        _built[0] = nc
    return _built[0]


def make_in_maps(inputs):
    import ml_dtypes

    bf16 = ml_dtypes.bfloat16
    f8 = ml_dtypes.float8_e4m3

    def f32(name):
        return np.ascontiguousarray(np.asarray(inputs[name], dtype=np.float32))

    inp = f32("inputs")
    h = f32("h")
    es = f32("encoder_states")
    ker = f32("kernel")
    rk = f32("recurrent_kernel").astype(bf16)
    bias = f32("bias")
    wa = f32("Wa")
    va = f32("Va")

    kernx = np.ascontiguousarray(ker[:XD]).astype(bf16)
    kernc = np.ascontiguousarray(ker[XD:] * KC_SCALE).astype(f8)

    # va2[p, i, q] = va[128*(2q+i)+p] * VA_SCALE  (cols q>=4 unused)
    va2 = np.zeros((128, 2, 16), np.float32)
    va2[:, :, 0:4] = (va[:, 0] * VA_SCALE).reshape(4, 2, 128).transpose(2, 1, 0)
    va2 = va2.astype(f8)

    wab = np.ascontiguousarray(wa[U:] * WA_SCALE).astype(f8)  # [1024, 1024]
    # wat [8u, 128p, 8j, 128c]: wat[u, p, j, :] = wa_top[128j+p, 128u:128u+128]
    wat = np.ascontiguousarray(
        wa[:U].astype(bf16).reshape(8, 128, 8, 128).transpose(2, 1, 0, 3)
    )

    in_maps = []
    for c in range(N_CORES):
        sl = slice(c * Bc, (c + 1) * Bc)
        es_c = es[sl].reshape(Bc * T, ED).astype(bf16)
        h_c = h[sl]
        # preshaped SBUF layouts: X.T [D, Bc] -> [128, D//128, Bc] (p, j, b)
        inT_c = inp[sl].T.reshape(4, 128, Bc).transpose(1, 0, 2)
        hT_c = h_c.T.reshape(8, 128, Bc).transpose(1, 0, 2)
        # es_pre [8g, 128p, 4r, 1024e]: es_pre[g, p, r] = es_c[512g+128r+p]
        es_pre = es_c.reshape(8, 4, 128, ED).transpose(0, 2, 1, 3).astype(f8)
        # esT_pre [8g, 128p, 8j, 512t]: esT_pre[g, p, j, t] = es_c[512g+t, 128j+p]
        esT_pre = np.ascontiguousarray(es_c.T).reshape(8, 128, 8, 512)
        esT_pre = esT_pre.transpose(2, 1, 0, 3).astype(f8)
        in_maps.append({
            "inT": np.ascontiguousarray(inT_c).astype(bf16),
            "h": h_c,
            "hT": np.ascontiguousarray(hT_c).astype(bf16),
            "es": np.ascontiguousarray(es_pre),
            "esT": np.ascontiguousarray(esT_pre),
            "kernx": kernx,
            "kernc": kernc,
            "rk": rk,
            "bias": bias,
            "wab": wab,
            "wat": wat,
            "va": va2,
        })
    return in_maps


def kernel(**inputs):
    nc = _get_nc()

    from concourse.bass_utils import run_bass_kernel_spmd

    in_maps = make_in_maps(inputs)
    res = run_bass_kernel_spmd(nc, in_maps, list(range(N_CORES)))
    out = np.concatenate(
        [res.results[c]["out"] for c in range(N_CORES)], axis=0
    ).astype(np.float32)
    return out


# revision 52
# speedup vs baseline: 1.0004x; 1.0004x over previous
"""Self-contained Trainium2 (Bass/Tile) kernel for the AttentionGRUCell
problem: 8-core data-parallel over batch, fp8/bf16 matmuls (<2e-2 rel err).

kernel(**inputs) takes the FULL unsharded inputs and returns the FULL
[512, 1088] output ([alpha, h_new] per row), running the Bass program on
NeuronCores 0-7 via run_bass_kernel_spmd.

Design notes (fp8 DoubleRow rewrite):
- The attention main GEMM tanh(es).T @ Wa_bot, the e-accumulation
  (Va . tanh(g)), the context matmul (alpha-masks @ es) and the GRU
  kernel ct-part streams all run as fp8(e4m3) DoubleRow matmuls: 256-row
  contraction per pass, ~1.2x the bf16 stream rate per instruction and
  half the instruction count. Host-side scales (Wa_bot x64, Va x16,
  kernel-ct x64, ct x8 on device) keep the fp8 encodings in the normal
  range; the scales are undone for free in the STT that folds the qk
  add / gate-bias adds (scalar=1/scale) and in the exp (scale=1/16).
- Error-critical paths stay bf16: qk (Wa_top), h@rk, inputs@kernel_x,
  (r*h)@rk_hh. hard_sigmoid's 0.2 slope and the smallness of the
  ct-part preactivation (std ~0.14) keep the fp8 error ~<0.5% of h_new.
- e-acc pairs lag half a superblock so the PE never waits on the
  STT+tanh chain; softmax/ct shift one superblock later (ct lag-1 kept).
- tesT/gT tanh are emitted in j-pairs ([128,1024] per ACT) to halve the
  ~350ns per-instruction ACT overhead.
- Startup: hT first, then esT0/wab chunk-interleaved so the first main
  matmuls start as soon as the first 256-row pair of tanh(es) lands.
- Tail: ct rows 0..55 transpose during g7; only the last 8 rows +
  copy-casts + DR streams remain after the loop, keeping the PE dense
  enough that the HAM clock stays warm.
"""
import sys

for _p in ("/opt/trn_rl_repo",):
    if _p not in sys.path:
        sys.path.insert(0, _p)

import numpy as np
import concourse.bass as bass
import concourse.mybir as mybir
import concourse.tile as tile
import bass_rust
from concourse.alu_op_type import AluOpType
from concourse.masks import make_identity
from concourse.vector_clock import ScopedClock

F32 = mybir.dt.float32
BF = mybir.dt.bfloat16
F8 = mybir.dt.float8e4
DR = mybir.MatmulPerfMode.DoubleRow
AF = mybir.ActivationFunctionType
AX = mybir.AxisListType

# host-side fp8 range scales, undone on device for free (STT scalars / exp
# scale).
WA_SCALE = 64.0   # Wa_bot
VA_SCALE = 16.0   # Va
KC_SCALE = 64.0   # kernel ct-part rows
CT_SCALE = 8.0    # ct (applied on device via srec8)

Bc, T, XD, ED, U = 64, 64, 512, 1024, 1024
NSBLK = 8
N_CORES = 8
B_FULL = 512


# ---------------------------------------------------------------------------
# Workarounds for this walrus build: instructions may carry at most one sem
# wait ("Too many sync wait commands"), including the Tile kernel-tail drain.
# ---------------------------------------------------------------------------

def _patched_drain_and_barrier(self, tick_clock, wait_clock):
    nc = self.nc
    probe = nc.sync.nop(nofuse=True)
    wait_clock.add_sem_waits(probe.ins, ScopedClock({None: tick_clock.global_clock}))
    si = probe.ins.sync_info
    waits = list(si.on_wait) if si is not None else []
    probe.ins.sync_info = bass_rust.SyncInfo(on_wait=waits[:1], on_update=[])
    for w in waits[1:]:
        n2 = nc.sync.nop(nofuse=True)
        n2.ins.sync_info = bass_rust.SyncInfo(on_wait=[w], on_update=[])
    nc.sync.drain()
    nc.all_engine_barrier()
    assert self.sems is not None
    popped = nc._tile_sem_poison_stack.pop()
    assert popped is self._sem_poison
    nc.clear_and_free_semaphores(list(self.sems.allocated().values()))
    nc.all_engine_barrier()


tile.TileContext._drain_and_barrier = _patched_drain_and_barrier

_fix_ctr = [0]


def fix_multi_waits(nc, max_waits=1):
    """Hoist extra sem waits onto same-engine InstNoOps placed immediately
    before the instruction -- engines execute in order, so semantics are
    identical."""
    for f in nc.m.functions:
        for blk in f.blocks:
            insts = blk.instructions
            if not any(
                i.sync_info is not None and len(i.sync_info.on_wait) > max_waits
                for i in insts
            ):
                continue
            out = []
            for inst in insts:
                si = inst.sync_info
                if si is not None and len(si.on_wait) > max_waits:
                    waits = list(si.on_wait)
                    for w in waits[:-max_waits]:
                        _fix_ctr[0] += 1
                        nop = mybir.InstNoOp(
                            name=f"waitfix-{_fix_ctr[0]}",
                            ins=[],
                            outs=[],
                            engine=inst.engine,
                        )
                        nop.sync_info = bass_rust.SyncInfo(on_wait=[w], on_update=[])
                        out.append(nop)
                    inst.sync_info = bass_rust.SyncInfo(
                        on_wait=waits[-max_waits:], on_update=list(si.on_update)
                    )
                out.append(inst)
            blk.instructions = out


# ---------------------------------------------------------------------------
# Kernel program
# ---------------------------------------------------------------------------

def build_nc():
    nc = bass.Bass("TRN2", target_bir_lowering=False, debug=False)

    inT_d = nc.dram_tensor("inT", [128, 4, Bc], BF, kind="ExternalInput")
    h_d = nc.dram_tensor("h", [Bc, U], F32, kind="ExternalInput")
    hT_d = nc.dram_tensor("hT", [128, 8, Bc], BF, kind="ExternalInput")
    es_d = nc.dram_tensor("es", [8, 128, 4, ED], F8, kind="ExternalInput")
    esT_d = nc.dram_tensor("esT", [8, 128, 8, 512], F8, kind="ExternalInput")
    kernx_d = nc.dram_tensor("kernx", [XD, 3 * U], BF, kind="ExternalInput")
    kernc_d = nc.dram_tensor("kernc", [ED, 3 * U], F8, kind="ExternalInput")
    rk_d = nc.dram_tensor("rk", [U, 3 * U], BF, kind="ExternalInput")
    bias_d = nc.dram_tensor("bias", [3 * U], F32, kind="ExternalInput")
    wa_d = nc.dram_tensor("wab", [ED, U], F8, kind="ExternalInput")
    wat_d = nc.dram_tensor("wat", [8, 128, 8, 128], BF, kind="ExternalInput")
    va_d = nc.dram_tensor("va", [128, 2, 16], F8, kind="ExternalInput")
    out_d = nc.dram_tensor("out", [Bc, T + U], F32, kind="ExternalOutput")

    with tile.TileContext(nc) as tc:
        with (
            tc.tile_pool(name="singles", bufs=1) as sg,
            tc.tile_pool(name="esT", bufs=2) as esT_pool,
            tc.tile_pool(name="esn", bufs=3) as esn_pool,
            tc.tile_pool(name="tesT", bufs=2) as tesT_pool,
            tc.tile_pool(name="gT", bufs=1) as gT_pool,
            tc.tile_pool(name="gst", bufs=4) as gst_pool,
            tc.tile_pool(name="wk", bufs=2) as wk_pool,
            tc.tile_pool(name="kx", bufs=2) as kx_pool,
            tc.tile_pool(name="kc", bufs=3) as kc_pool,
            tc.tile_pool(name="smalls", bufs=4) as sm_pool,
            tc.tile_pool(name="ps_v", bufs=3, space="PSUM") as ps_v,
            tc.tile_pool(name="ps_tr", bufs=1, space="PSUM") as ps_tr,
            tc.tile_pool(name="ps_acc", bufs=2, space="PSUM") as ps_acc,
            tc.tile_pool(name="ps_e", bufs=1, space="PSUM") as ps_e,
            tc.tile_pool(name="ps_ct", bufs=1, space="PSUM") as ps_ct,
        ):
            # ---- startup DMAs, spread over engine queues: each dma_start
            # costs ~640ns of enqueue time on its issuing engine, so the
            # critical esT0/wab chunks get the sync queue to themselves ----
            hT_sb = sg.tile([128, 8, Bc], BF)
            nc.gpsimd.dma_start(out=hT_sb[:], in_=hT_d[:])

            # esT0 / wab chunk-interleaved: the first DR main matmul only
            # needs pair 0 of both.
            esT_cur = esT_pool.tile([128, 8, 512], F8, tag="esT", name="esT0")
            wab_sb = sg.tile([128, 8, U], F8)
            # first column-half of wab only: mains u0..u3 need wab cols
            # 0:512, so the critical startup prefix is 1MB, not 1.5MB
            for q in range(4):
                nc.sync.dma_start(
                    out=esT_cur[:, 2 * q:2 * q + 2, :],
                    in_=esT_d.ap()[0, :, 2 * q:2 * q + 2, :],
                )
                nc.sync.dma_start(
                    out=wab_sb[:, 2 * q:2 * q + 2, 0:512],
                    in_=wa_d.ap()[256 * q:256 * (q + 1), 0:512].rearrange(
                        "(i p) c -> p i c", i=2
                    ),
                )
            for q in range(4):
                nc.sync.dma_start(
                    out=wab_sb[:, 2 * q:2 * q + 2, 512:1024],
                    in_=wa_d.ap()[256 * q:256 * (q + 1), 512:1024].rearrange(
                        "(i p) c -> p i c", i=2
                    ),
                )

            va_sb = sg.tile([128, 2, 16], F8)
            nc.gpsimd.dma_start(out=va_sb[:], in_=va_d[:])
            # qk weight blocks: two enqueues on the sync queue AFTER the
            # critical esT0/wab chunks (priority = sync enqueue order)
            wau_all = sg.tile([128, 8, 8, 128], BF)
            for uh in range(2):
                nc.sync.dma_start(
                    out=wau_all[:, 4 * uh:4 * (uh + 1), :, :],
                    in_=wat_d.ap()[4 * uh:4 * (uh + 1)].rearrange(
                        "u p d c -> p u d c"
                    ),
                )
            h_sb = sg.tile([Bc, U], F32)
            nc.sync.dma_start(out=h_sb[:], in_=h_d[:])
            inT_sb = sg.tile([128, 4, Bc], BF)
            nc.sync.dma_start(out=inT_sb[:], in_=inT_d[:])

            def load_esT(g):
                t_ = esT_pool.tile([128, 8, 512], F8, tag="esT", name=f"esT{g}")
                nc.sync.dma_start(out=t_[:], in_=esT_d.ap()[g])
                return t_

            esn_tiles = {}

            def load_esn(g):
                t_ = esn_pool.tile([128, 4, ED], F8, tag="esn", name=f"esn{g}")
                nc.sync.dma_start(out=t_[:], in_=es_d.ap()[g])
                esn_tiles[g] = t_

            # warm the ACT table while the first DMAs are in flight
            scr1 = sg.tile([1, 1], F32)
            nc.vector.memset(scr1[:], 0.0)
            nc.scalar.activation(out=scr1[:], in_=scr1[:], func=AF.Tanh)

            bias_b = sg.tile([Bc, 3 * U], F32)
            # one partition-broadcast enqueue for the whole bias
            _bsrc = bias_d.ap()
            nc.gpsimd.dma_start(
                out=bias_b[:],
                in_=bass.AP(
                    tensor=_bsrc.tensor,
                    offset=_bsrc.offset,
                    ap=[[0, Bc], [1, 3 * U]],
                ),
            )

            esT_tiles = {0: esT_cur}

            thT = sg.tile([128, 8, Bc], BF)

            ident = sg.tile([16, 16], F32)
            make_identity(nc, ident[:])
            identb = sg.tile([Bc, Bc], BF)
            nc.vector.memset(identb[:], 0.0)
            nc.gpsimd.affine_select(
                out=identb[:],
                in_=identb[:],
                compare_op=mybir.AluOpType.not_equal,
                fill=1.0,
                base=0,
                pattern=[[-1, Bc]],
                channel_multiplier=1,
            )

            masks = sg.tile([128, 4, 8], F32)
            nc.vector.memset(masks[:], 0.0)
            for rr in range(4):
                nc.vector.memset(masks[0:64, rr, 2 * rr:2 * rr + 1], 1.0)
                nc.vector.memset(masks[64:128, rr, 2 * rr + 1:2 * rr + 2], 1.0)

            half_sb = sg.tile([Bc, 1], F32)
            nc.vector.memset(half_sb[:], 0.5)

            qkT = sg.tile([128, 8, Bc], BF)
            ctT = sg.tile([128, 8, Bc], F8)
            gh_sb = sg.tile([Bc, 2 * U], BF)
            gx0_sb = sg.tile([Bc, 3 * U], BF)
            ct_sb = sg.tile([Bc, ED], BF)       # rows 0..55 used (g0..g6)
            ct_sb7 = sg.tile([8, ED], BF)       # g7's rows (base-0 partitions)

            def qk_block_u(u):
                # qkT[u-block] = (Wa_top.T @ tanh(h).T) via small direct matmuls
                pq = ps_tr.tile([128, Bc], F32, tag="tr", name=f"pq{u}")
                for d in range(8):
                    nc.tensor.matmul(
                        pq[:],
                        wau_all[:, u, d, :],
                        thT[:, d, :],
                        start=(d == 0),
                        stop=(d == 7),
                    )
                nc.vector.tensor_copy(qkT[:, u, :], pq[:])

            # --- GRU weight-stream pieces: one batched enqueue per 512-col
            # slab, consumed d-granular by matmuls in the u-loop ---
            rkz_tiles = {}

            def load_rkz(n):
                # two half-slab enqueues so d=0 matmuls don't wait the full MB
                t_ = wk_pool.tile([128, 8, 512], BF, tag="wk", name=f"rkz{n}")
                for dh in range(2):
                    nc.sync.dma_start(
                        out=t_[:, 4 * dh:4 * (dh + 1), :],
                        in_=rk_d.ap()[
                            512 * dh:512 * (dh + 1), 512 * n:512 * (n + 1)
                        ].rearrange("(d p) c -> p d c", d=4),
                    )
                rkz_tiles[n] = t_

            def gh_step(n, d, pg):
                nc.tensor.matmul(
                    pg[:], hT_sb[:, d, :], rkz_tiles[n][:, d, :],
                    start=(d == 0), stop=(d == 7),
                )

            def gh_fin(n, pg):
                nc.vector.scalar_tensor_tensor(
                    out=gh_sb[:, 512 * n:512 * (n + 1)],
                    in0=pg[:],
                    scalar=1.0,
                    in1=bias_b[:, 512 * n:512 * (n + 1)],
                    op0=AluOpType.mult,
                    op1=AluOpType.add,
                )

            kx_tiles = {}

            def load_kx(n):
                t_ = kx_pool.tile([128, 4, 512], BF, tag="kx", name=f"kx{n}")
                for dh in range(2):
                    nc.sync.dma_start(
                        out=t_[:, 2 * dh:2 * (dh + 1), :],
                        in_=kernx_d.ap()[
                            256 * dh:256 * (dh + 1), 512 * n:512 * (n + 1)
                        ].rearrange("(d p) c -> p d c", d=2),
                    )
                kx_tiles[n] = t_

            def gx0_step(n, d, pa):
                nc.tensor.matmul(
                    pa[:], inT_sb[:, d, :], kx_tiles[n][:, d, :],
                    start=(d == 0), stop=(d == 3),
                )

            # --- pipelined e-acc -> alpha -> c_t pieces (lagged) ---
            pe_tiles = {}

            def eacc_pair(g, q, gT_g):
                # e[g] += (va_q).T @ gT[pair q]  -- fp8 DoubleRow, K=256
                if q == 0:
                    pe_tiles[g] = ps_e.tile([1, 512], F32, tag="e", name=f"pe{g}")
                nc.tensor.matmul(
                    pe_tiles[g][:],
                    va_sb[:, :, q:q + 1],
                    gT_g[:, 2 * q:2 * q + 2, :],
                    start=(q == 0),
                    stop=(q == 3),
                    perf_mode=DR,
                )

            def softmax(g):
                # alpha stays UNNORMALIZED; ct is scaled by CT_SCALE/sum via
                # srec8. exp undoes the VA_SCALE on e for free.
                e_sb = sm_pool.tile([1, 512], F32, tag="esb", name=f"esb{g}", bufs=1)
                nc.vector.tensor_copy(e_sb[:], pe_tiles[g][:])
                alpha = sm_pool.tile([8, T], F32, tag="al", name=f"al{g}")
                # scalar queue: hardware-dynamic and nearly empty, so this
                # latency-critical reshape never starves behind the bulk
                # weight slabs (gpsimd's software queue stalls ~7us at g6/g7)
                nc.scalar.dma_start(
                    out=alpha[:],
                    in_=e_sb[0:1, :].rearrange("p (b t) -> p b t", b=8),
                )
                ssum = sm_pool.tile([8, 1], F32, tag="ssum", name=f"ss{g}")
                nc.scalar.activation(
                    out=alpha[:], in_=alpha[:], func=AF.Exp,
                    scale=1.0 / VA_SCALE, accum_out=ssum[:],
                )
                srec = sm_pool.tile([8, 1], F32, tag="srec", name=f"sr{g}")
                nc.vector.reciprocal(srec[:], ssum[:])
                srec8 = sm_pool.tile([8, 1], F32, tag="srec8", name=f"sr8{g}")
                nc.vector.tensor_scalar_mul(srec8[:], srec[:], CT_SCALE)
                alpha_n = sm_pool.tile([8, T], F32, tag="aln", name=f"aln{g}")
                nc.vector.tensor_scalar_mul(alpha_n[:], alpha[:], srec[:])
                nc.gpsimd.dma_start(
                    out=out_d.ap()[8 * g:8 * (g + 1), 0:T], in_=alpha_n[:]
                )
                return alpha, srec8

            def ct_head(g, alpha):
                pat = ps_ct.tile([T, 8], F32, tag="ct", name=f"pat{g}")
                nc.tensor.transpose(pat[:], alpha[:], ident[:8, :8])
                alpT2 = sm_pool.tile([128, 8], F32, tag="alT2", name=f"aT2{g}")
                nc.vector.tensor_copy(alpT2[0:64, :], pat[:])
                nc.vector.tensor_copy(alpT2[64:128, :], pat[:])
                a2s = []
                for q in range(2):
                    a2 = sm_pool.tile([128, 2, 16], F8, tag="A", name=f"A{g}_{q}")
                    for i in range(2):
                        nc.gpsimd.tensor_mul(
                            a2[:, i, 0:8], alpT2[:], masks[:, 2 * q + i, :]
                        )
                    a2s.append(a2)
                return a2s

            def ct_tail(g, esn_g, a2s, srec8_g):
                # fp8 DR: out[8b, 512e] accumulating rr-pairs; g7's rows land
                # in the base-0 ct_sb7 tile directly, earlier rows go through
                # a staging tile + DMA (DVE writes need 32-aligned partition
                # bases).
                if g == 7:
                    stage = ct_sb7
                else:
                    stage = sm_pool.tile(
                        [8, ED], BF, tag="ctst", name=f"cts{g}", bufs=1
                    )
                for n in range(2):
                    pct = ps_ct.tile([8, 512], F32, tag="ct", name=f"pct{g}_{n}")
                    for q in range(2):
                        nc.tensor.matmul(
                            pct[:],
                            a2s[q][:, :, 0:8],
                            esn_g[:, 2 * q:2 * q + 2, 512 * n:512 * (n + 1)],
                            start=(q == 0),
                            stop=(q == 1),
                            perf_mode=DR,
                        )
                    nc.vector.tensor_scalar_mul(
                        stage[:, 512 * n:512 * (n + 1)], pct[:], srec8_g[:]
                    )
                if g != 7:
                    nc.gpsimd.dma_start(
                        out=ct_sb[8 * g:8 * (g + 1), :], in_=stage[:]
                    )

            # GRU ct-part weight preloads (fp8): one enqueue per gate part,
            # shaped [128, 4dcp, 2i, 1024] so DR pair slices come out 3D.
            kc_tiles = {}

            def preload_tail_part(part):
                wp = kc_pool.tile(
                    [128, 4, 2, U], F8, tag="kc", name=f"kc{part}"
                )
                nc.sync.dma_start(
                    out=wp[:],
                    in_=kernc_d.ap()[:, U * part:U * (part + 1)].rearrange(
                        "(q i p) c -> p q i c", q=4, i=2
                    ),
                )
                kc_tiles[part] = wp

            def emit_tesT_tanh(dst, src):
                for q in range(4):
                    nc.scalar.activation(
                        out=dst[:, 2 * q:2 * q + 2, :],
                        in_=src[:, 2 * q:2 * q + 2, :],
                        func=AF.Tanh,
                    )

            rkh_all = sg.tile([128, 8, U], BF)

            _dummy_ctr = [0]

            def dummy_mm(k):
                # dep-free 64-col transposes: ~150ns each of PE activity to
                # keep the HAM clock warm across short serial waits
                for _ in range(k):
                    _dummy_ctr[0] += 1
                    pdm = ps_tr.tile(
                        [Bc, Bc], BF, tag="tr", name=f"dum{_dummy_ctr[0]}"
                    )
                    nc.tensor.transpose(pdm[:], identb[:], identb[:])

            # ---- attention superblock loop ----
            load_rkz(0)
            tesT_cur = tesT_pool.tile([128, 8, 512], F8, tag="tesT", name="tes0")
            emit_tesT_tanh(tesT_cur, esT_cur)
            nc.scalar.activation(out=thT[:], in_=hT_sb[:], func=AF.Tanh)

            gT_tiles = {}
            alpha_info = {}   # g -> (alpha, srec8)
            a2s_prev = None

            for g in range(NSBLK):
                gh_n = g - 1 if 1 <= g <= 4 else None
                # hh-column chunks (n=4,5) move to the post-loop tail as
                # g7-independent PE filler
                gx_n = g - 2 if 2 <= g <= 5 else None
                pg = (
                    ps_acc.tile([Bc, 512], F32, tag="acc", name=f"pg{gh_n}")
                    if gh_n is not None else None
                )
                pa = (
                    ps_acc.tile([Bc, 512], F32, tag="acc", name=f"gx0_{gx_n}")
                    if gx_n is not None else None
                )

                tesT = tesT_cur
                ilv = 1 <= g < NSBLK - 1   # interleave next tesT tanh (g>=1)
                if ilv:
                    tesT_nxt = tesT_pool.tile(
                        [128, 8, 512], F8, tag="tesT", name=f"tes{g+1}"
                    )
                gT = gT_pool.tile([128, 8, 512], F8, tag="gT", name=f"gt{g}")
                gT_tiles[g] = gT
                gstage = gst_pool.tile(
                    [128, 2, 512], BF, tag="gst", name=f"gs{g}"
                )
                for u in range(8):
                    pv = ps_v.tile([128, 512], F32, tag="v", name=f"pv{g}_{u}")
                    for j2 in range(4):
                        # fp8 DoubleRow: contracts 256 rows per pass
                        nc.tensor.matmul(
                            pv[:],
                            wab_sb[:, 2 * j2:2 * j2 + 2, 128 * u:128 * (u + 1)],
                            tesT[:, 2 * j2:2 * j2 + 2, :],
                            start=(j2 == 0),
                            stop=(j2 == 3),
                            perf_mode=DR,
                        )
                    if g == 0:
                        # must precede the STT below, which reads qkT[:, u, :]
                        qk_block_u(u)
                    qk_slice = qkT[:, u, 8 * g:8 * g + 8]
                    qk_bc = bass.AP(
                        tensor=qk_slice.tensor,
                        offset=qk_slice.offset,
                        ap=[
                            list(qk_slice.ap[0]),
                            list(qk_slice.ap[1]),
                            [0, T],
                        ],
                    )
                    nc.vector.scalar_tensor_tensor(
                        out=gstage[:, u % 2, :],
                        in0=pv[:],
                        scalar=1.0 / WA_SCALE,
                        in1=qk_bc,
                        op0=AluOpType.mult,
                        op1=AluOpType.add,
                    )
                    if u % 2 == 1:
                        # batched pair tanh -> fp8 gT
                        nc.scalar.activation(
                            out=gT[:, u - 1:u + 1, :],
                            in_=gstage[:],
                            func=AF.Tanh,
                        )
                        gstage = gst_pool.tile(
                            [128, 2, 512], BF, tag="gst", name=f"gs{g}_{u}"
                        )
                        if ilv:
                            # next superblock's tesT tanh pair
                            qq = (u - 1) // 2
                            nc.scalar.activation(
                                out=tesT_nxt[:, 2 * qq:2 * qq + 2, :],
                                in_=esT_tiles[g + 1][:, 2 * qq:2 * qq + 2, :],
                                func=AF.Tanh,
                            )
                    # lagged e-acc pairs: q0/q1 of g at u=5/7, q2/q3 of g-1
                    # at u=0/2. softmax starts at u=2 and ct_head waits to
                    # u=6 so the alpha-reshape DMA's software-queue latency
                    # jitter (multi-us) never stalls the PE at pat.
                    if u == 5:
                        eacc_pair(g, 0, gT)
                    elif u == 7:
                        eacc_pair(g, 1, gT)
                    elif u == 0 and g >= 1:
                        eacc_pair(g - 1, 2, gT_tiles[g - 1])
                    elif u == 2 and g >= 1:
                        eacc_pair(g - 1, 3, gT_tiles[g - 1])
                    if u == 2 and g >= 1:
                        alpha_info[g - 1] = softmax(g - 1)
                    if u == 5 and g >= 1:
                        a2s_prev = ct_head(g - 1, alpha_info[g - 1][0])
                    if u == 7 and g >= 1:
                        ct_tail(
                            g - 1, esn_tiles[g - 1], a2s_prev,
                            alpha_info[g - 1][1],
                        )
                    if gh_n is not None:
                        gh_step(gh_n, u, pg)
                    if gx_n is not None and u < 4:
                        gx0_step(gx_n, u, pa)

                if gh_n is not None:
                    gh_fin(gh_n, pg)
                if gx_n is not None:
                    # fold gh (z/r cols) into the gx0 store so the GRU tail
                    # does a single add per gate column
                    nc.vector.scalar_tensor_tensor(
                        out=gx0_sb[:, 512 * gx_n:512 * (gx_n + 1)],
                        in0=pa[:],
                        scalar=1.0,
                        in1=gh_sb[:, 512 * gx_n:512 * (gx_n + 1)],
                        op0=AluOpType.mult,
                        op1=AluOpType.add,
                    )

                if ilv:
                    tesT_cur = tesT_nxt
                elif g == 0:
                    # g0: batch-tanh tesT(1) after the u-loop (esT1 lands late)
                    esT_tiles[1] = load_esT(1)
                    tesT_cur = tesT_pool.tile(
                        [128, 8, 512], F8, tag="tesT", name="tes1"
                    )
                    emit_tesT_tanh(tesT_cur, esT_tiles[1])
                if g + 2 < NSBLK:
                    esT_tiles[g + 2] = load_esT(g + 2)

                # spread remaining weight streams across the attention phase
                if g == 0:
                    load_esn(0)
                    load_esn(1)
                    load_esn(2)
                elif g + 2 < NSBLK:
                    load_esn(g + 2)
                if 1 <= g <= 3:
                    load_rkz(g)      # h@rk slab for gh at g+1
                if 1 <= g <= 6:
                    load_kx(g - 1)   # inputs@kernel slab for gx0 at g+1
                if g == 5:
                    preload_tail_part(0)
                if g == 6:
                    preload_tail_part(1)
                    nc.sync.dma_start(
                        out=rkh_all[:],
                        in_=rk_d.ap()[:, 2 * U:].rearrange(
                            "(d p) c -> p d c", d=8
                        ),
                    )
                if g == 7:
                    preload_tail_part(2)

            # ---- post-loop: finish g7's e/alpha/ct; keep the PE dense with
            # g7-independent filler so the HAM clock never drops ----
            eacc_pair(7, 2, gT_tiles[7])
            eacc_pair(7, 3, gT_tiles[7])
            alpha_info[7] = softmax(7)
            dummy_mm(4)

            # real filler: inputs-part gx0 for the hh columns (n=4,5)
            for n45 in (4, 5):
                pa45 = ps_acc.tile([Bc, 512], F32, tag="acc", name=f"gx0_{n45}")
                for d in range(4):
                    gx0_step(n45, d, pa45)
                nc.vector.scalar_tensor_tensor(
                    out=gx0_sb[:, 512 * n45:512 * (n45 + 1)],
                    in0=pa45[:],
                    scalar=1.0,
                    in1=bias_b[:, 512 * n45:512 * (n45 + 1)],
                    op0=AluOpType.mult,
                    op1=AluOpType.add,
                )

            # pre-transpose ct rows 0..55 (g0..g6) into fp8 ctT -- fills the
            # PE while g7's softmax chain runs on gpsimd/ACT/DVE
            for j in range(8):
                pool, tag = (ps_tr, "tr") if j % 2 == 0 else (ps_ct, "ct")
                pt = pool.tile([128, 56], BF, tag=tag, name=f"tpc_{j}")
                nc.tensor.transpose(
                    pt[:], ct_sb[0:56, 128 * j:128 * (j + 1)],
                    identb[0:56, 0:56],
                )
                nc.scalar.copy(ctT[:, j, 0:56], pt[:])
            dummy_mm(4)
            a2s_prev = ct_head(7, alpha_info[7][0])
            dummy_mm(3)
            ct_tail(7, esn_tiles[7], a2s_prev, alpha_info[7][1])
            dummy_mm(4)

            # ---- GRU tail ----
            z_sb = sg.tile([Bc, U], F32)
            r_sb = sg.tile([Bc, U], F32)
            rh_bf = sg.tile([Bc, U], BF)
            rhT = sg.tile([128, 8, Bc], BF)
            zh_sb = sg.tile([Bc, U], F32)
            omz_sb = sg.tile([Bc, U], F32)
            hh_sb = r_sb  # r is dead once rh_bf is formed

            # six gate accumulators in the (now idle) attention psum banks
            gx = [
                ps_v.tile([Bc, 512], F32, tag="v", name="gxa"),
                ps_v.tile([Bc, 512], F32, tag="v", name="gxb"),
                ps_v.tile([Bc, 512], F32, tag="v", name="gxc"),
                ps_e.tile([Bc, 512], F32, tag="e", name="gxd"),
                ps_acc.tile([Bc, 512], F32, tag="acc", name="gxe"),
                ps_acc.tile([Bc, 512], F32, tag="acc", name="gxf"),
            ]

            # last 8 ct rows: transpose + copy-cast
            for j in range(8):
                pool, tag = (ps_tr, "tr") if j % 2 == 0 else (ps_ct, "ct")
                pt = pool.tile([128, 8], BF, tag=tag, name=f"tp8_{j}")
                nc.tensor.transpose(
                    pt[:], ct_sb7[:, 128 * j:128 * (j + 1)], identb[0:8, 0:8]
                )
                nc.scalar.copy(ctT[:, j, 56:64], pt[:])
            dummy_mm(3)

            # z/r ct-part: fp8 DR pairs (scale 1/(KC*CT) folded into the
            # adds). r gates (part 1) stream FIRST: their psums gate the
            # rhT transposes, so finishing them early overlaps the r-gate
            # chain with the z and hh-ct streams.
            for part in (1, 0):
                for dcp in range(4):
                    for half in range(2):
                        n = 2 * part + half
                        nc.tensor.matmul(
                            gx[n][:],
                            ctT[:, 2 * dcp:2 * dcp + 2, :],
                            kc_tiles[part][:, dcp, :, 512 * half:512 * (half + 1)],
                            start=(dcp == 0),
                            stop=(dcp == 3),
                            perf_mode=DR,
                        )

            GATE_RS = 1.0 / (KC_SCALE * CT_SCALE)

            def add_inplace(pa_, n, src_sb, scalar):
                nc.vector.scalar_tensor_tensor(
                    out=pa_[:],
                    in0=pa_[:],
                    scalar=scalar,
                    in1=src_sb[:, 512 * n:512 * (n + 1)],
                    op0=AluOpType.mult,
                    op1=AluOpType.add,
                )

            # hh ct-part stream first (keeps PE busy while the z/r gate chain
            # runs on ACT/DVE); rh contributions are pre-scaled x512 so the
            # shared psum stays on one scale
            for dcp in range(4):
                for half in range(2):
                    nc.tensor.matmul(
                        gx[4 + half][:],
                        ctT[:, 2 * dcp:2 * dcp + 2, :],
                        kc_tiles[2][:, dcp, :, 512 * half:512 * (half + 1)],
                        start=(dcp == 0),
                        stop=False,
                        perf_mode=DR,
                    )

            # gates: hard_sigmoid(gx + gx0 + gh) = min(relu(0.2x+0.5), 1).
            # r gates FIRST -- only r gates the PE-critical rhT transposes;
            # z feeds nothing until the final combine.
            dummy_mm(6)
            for n in (2, 3, 0, 1):
                dst = z_sb if n < 2 else r_sb
                o = 512 * (n % 2)
                sl = slice(o, o + 512)
                add_inplace(gx[n], n, gx0_sb, GATE_RS)
                nc.scalar.activation(
                    out=dst[:, sl], in_=gx[n][:],
                    func=AF.Relu, bias=half_sb[:], scale=0.2,
                )
                nc.vector.tensor_scalar_min(dst[:, sl], dst[:, sl], 1.0)
                if n < 2:
                    # precompute z*h and (1-z) off the critical path
                    nc.vector.tensor_mul(zh_sb[:, sl], dst[:, sl], h_sb[:, sl])
                    nc.vector.tensor_scalar(
                        out=omz_sb[:, sl], in0=dst[:, sl],
                        scalar1=-1.0, scalar2=1.0,
                        op0=AluOpType.mult, op1=AluOpType.add,
                    )
                else:
                    # rh = (r * 512) * h so the hh psum (ct-part x512) stays
                    # single-scale
                    nc.vector.scalar_tensor_tensor(
                        out=rh_bf[:, sl],
                        in0=dst[:, sl],
                        scalar=1.0 / GATE_RS,
                        in1=h_sb[:, sl],
                        op0=AluOpType.mult,
                        op1=AluOpType.mult,
                    )
                    for j in range(4 * (n - 2), 4 * (n - 1)):
                        pool, tag = (ps_tr, "tr") if j % 2 == 0 else (ps_ct, "ct")
                        pt = pool.tile([128, Bc], BF, tag=tag, name=f"tprh_{j}")
                        nc.tensor.transpose(
                            pt[:], rh_bf[:, 128 * j:128 * (j + 1)], identb[:]
                        )
                        nc.scalar.copy(rhT[:, j, :], pt[:])

            # (r*h) @ rk_hh stream, bank-major so the first hh half finishes
            # early and its scalar/vector chain overlaps the second bank
            for n2 in range(2):
                for d in range(8):
                    nc.tensor.matmul(
                        gx[4 + n2][:],
                        rhT[:, d, :],
                        rkh_all[:, d, 512 * n2:512 * (n2 + 1)],
                        start=False,
                        stop=(d == 7),
                    )

            # hh = tanh(...); h_new = z*h + (1-z)*hh  (zh/omz precomputed)
            t1 = sg.tile([Bc, U], F32)
            for n2 in range(2):
                o = 512 * n2
                sl = slice(o, o + 512)
                pa = gx[4 + n2]
                add_inplace(pa, 4 + n2, gx0_sb, GATE_RS)
                nc.scalar.activation(out=hh_sb[:, sl], in_=pa[:], func=AF.Tanh)
                nc.vector.tensor_mul(t1[:, sl], omz_sb[:, sl], hh_sb[:, sl])
                nc.vector.tensor_add(t1[:, sl], t1[:, sl], zh_sb[:, sl])
                nc.sync.dma_start(out=out_d.ap()[:, T + o:T + o + 512], in_=t1[:, sl])

    return nc


_built = [None]


def _get_nc():
    if _built[0] is None:
        nc = build_nc()
        fix_multi_waits(nc)
        _built[0] = nc
    return _built[0]


def make_in_maps(inputs):
    import ml_dtypes

    bf16 = ml_dtypes.bfloat16
    f8 = ml_dtypes.float8_e4m3

    def f32(name):
        return np.ascontiguousarray(np.asarray(inputs[name], dtype=np.float32))

    inp = f32("inputs")
    h = f32("h")
    es = f32("encoder_states")
    ker = f32("kernel")
    rk = f32("recurrent_kernel").astype(bf16)
    bias = f32("bias")
    wa = f32("Wa")
    va = f32("Va")

    kernx = np.ascontiguousarray(ker[:XD]).astype(bf16)
    kernc = np.ascontiguousarray(ker[XD:] * KC_SCALE).astype(f8)

    # va2[p, i, q] = va[128*(2q+i)+p] * VA_SCALE  (cols q>=4 unused)
    va2 = np.zeros((128, 2, 16), np.float32)
    va2[:, :, 0:4] = (va[:, 0] * VA_SCALE).reshape(4, 2, 128).transpose(2, 1, 0)
    va2 = va2.astype(f8)

    wab = np.ascontiguousarray(wa[U:] * WA_SCALE).astype(f8)  # [1024, 1024]
    # wat [8u, 128p, 8j, 128c]: wat[u, p, j, :] = wa_top[128j+p, 128u:128u+128]
    wat = np.ascontiguousarray(
        wa[:U].astype(bf16).reshape(8, 128, 8, 128).transpose(2, 1, 0, 3)
    )

    in_maps = []
    for c in range(N_CORES):
        sl = slice(c * Bc, (c + 1) * Bc)
        es_c = es[sl].reshape(Bc * T, ED).astype(bf16)
        h_c = h[sl]
        # preshaped SBUF layouts: X.T [D, Bc] -> [128, D//128, Bc] (p, j, b)
        inT_c = inp[sl].T.reshape(4, 128, Bc).transpose(1, 0, 2)
        hT_c = h_c.T.reshape(8, 128, Bc).transpose(1, 0, 2)
        # es_pre [8g, 128p, 4r, 1024e]: es_pre[g, p, r] = es_c[512g+128r+p]
        es_pre = es_c.reshape(8, 4, 128, ED).transpose(0, 2, 1, 3).astype(f8)
        # esT_pre [8g, 128p, 8j, 512t]: esT_pre[g, p, j, t] = es_c[512g+t, 128j+p]
        esT_pre = np.ascontiguousarray(es_c.T).reshape(8, 128, 8, 512)
        esT_pre = esT_pre.transpose(2, 1, 0, 3).astype(f8)
        in_maps.append({
            "inT": np.ascontiguousarray(inT_c).astype(bf16),
            "h": h_c,
            "hT": np.ascontiguousarray(hT_c).astype(bf16),
            "es": np.ascontiguousarray(es_pre),
            "esT": np.ascontiguousarray(esT_pre),
            "kernx": kernx,
            "kernc": kernc,
            "rk": rk,
            "bias": bias,
            "wab": wab,
            "wat": wat,
            "va": va2,
        })
    return in_maps


def kernel(**inputs):
    nc = _get_nc()

    from concourse.bass_utils import run_bass_kernel_spmd

    in_maps = make_in_maps(inputs)
    res = run_bass_kernel_spmd(nc, in_maps, list(range(N_CORES)))
    out = np.concatenate(
        [res.results[c]["out"] for c in range(N_CORES)], axis=0
    ).astype(np.float32)
    return out


# revision 53
# speedup vs baseline: 1.1582x; 1.1577x over previous
"""Self-contained Trainium2 (Bass/Tile) kernel for the AttentionGRUCell
problem: 8-core data-parallel over batch, fp8/bf16 matmuls (<2e-2 rel err).

kernel(**inputs) takes the FULL unsharded inputs and returns the FULL
[512, 1088] output ([alpha, h_new] per row), running the Bass program on
NeuronCores 0-7 via run_bass_kernel_spmd.

Design notes (fp8 DoubleRow rewrite):
- The attention main GEMM tanh(es).T @ Wa_bot, the e-accumulation
  (Va . tanh(g)), the context matmul (alpha-masks @ es) and the GRU
  kernel ct-part streams all run as fp8(e4m3) DoubleRow matmuls: 256-row
  contraction per pass, ~1.2x the bf16 stream rate per instruction and
  half the instruction count. Host-side scales (Wa_bot x64, Va x16,
  kernel-ct x64, ct x8 on device) keep the fp8 encodings in the normal
  range; the scales are undone for free in the STT that folds the qk
  add / gate-bias adds (scalar=1/scale) and in the exp (scale=1/16).
- Error-critical paths stay bf16: qk (Wa_top), h@rk, inputs@kernel_x,
  (r*h)@rk_hh. hard_sigmoid's 0.2 slope and the smallness of the
  ct-part preactivation (std ~0.14) keep the fp8 error ~<0.5% of h_new.
- e-acc pairs lag half a superblock so the PE never waits on the
  STT+tanh chain; softmax/ct shift one superblock later (ct lag-1 kept).
- tesT/gT tanh are emitted in j-pairs ([128,1024] per ACT) to halve the
  ~350ns per-instruction ACT overhead.
- Startup: hT first, then esT0/wab chunk-interleaved so the first main
  matmuls start as soon as the first 256-row pair of tanh(es) lands.
- Tail: ct rows 0..55 transpose during g7; only the last 8 rows +
  copy-casts + DR streams remain after the loop, keeping the PE dense
  enough that the HAM clock stays warm.
"""
import sys

for _p in ("/opt/trn_rl_repo",):
    if _p not in sys.path:
        sys.path.insert(0, _p)

import numpy as np
import concourse.bass as bass
import concourse.mybir as mybir
import concourse.tile as tile
import bass_rust
from concourse.alu_op_type import AluOpType
from concourse.masks import make_identity
from concourse.vector_clock import ScopedClock

F32 = mybir.dt.float32
BF = mybir.dt.bfloat16
F8 = mybir.dt.float8e4
DR = mybir.MatmulPerfMode.DoubleRow
AF = mybir.ActivationFunctionType
AX = mybir.AxisListType

# host-side fp8 range scales, undone on device for free (STT scalars / exp
# scale).
WA_SCALE = 64.0   # Wa_bot
VA_SCALE = 16.0   # Va
KC_SCALE = 64.0   # kernel ct-part rows
CT_SCALE = 8.0    # ct (applied on device via srec8)

Bc, T, XD, ED, U = 64, 64, 512, 1024, 1024
NSBLK = 8
N_CORES = 8
B_FULL = 512


# ---------------------------------------------------------------------------
# Workarounds for this walrus build: instructions may carry at most one sem
# wait ("Too many sync wait commands"), including the Tile kernel-tail drain.
# ---------------------------------------------------------------------------

def _patched_drain_and_barrier(self, tick_clock, wait_clock):
    nc = self.nc
    probe = nc.sync.nop(nofuse=True)
    wait_clock.add_sem_waits(probe.ins, ScopedClock({None: tick_clock.global_clock}))
    si = probe.ins.sync_info
    waits = list(si.on_wait) if si is not None else []
    probe.ins.sync_info = bass_rust.SyncInfo(on_wait=waits[:1], on_update=[])
    for w in waits[1:]:
        n2 = nc.sync.nop(nofuse=True)
        n2.ins.sync_info = bass_rust.SyncInfo(on_wait=[w], on_update=[])
    nc.sync.drain()
    nc.all_engine_barrier()
    assert self.sems is not None
    popped = nc._tile_sem_poison_stack.pop()
    assert popped is self._sem_poison
    nc.clear_and_free_semaphores(list(self.sems.allocated().values()))
    nc.all_engine_barrier()


tile.TileContext._drain_and_barrier = _patched_drain_and_barrier

_fix_ctr = [0]


def fix_multi_waits(nc, max_waits=1):
    """Hoist extra sem waits onto same-engine InstNoOps placed immediately
    before the instruction -- engines execute in order, so semantics are
    identical."""
    for f in nc.m.functions:
        for blk in f.blocks:
            insts = blk.instructions
            if not any(
                i.sync_info is not None and len(i.sync_info.on_wait) > max_waits
                for i in insts
            ):
                continue
            out = []
            for inst in insts:
                si = inst.sync_info
                if si is not None and len(si.on_wait) > max_waits:
                    waits = list(si.on_wait)
                    for w in waits[:-max_waits]:
                        _fix_ctr[0] += 1
                        nop = mybir.InstNoOp(
                            name=f"waitfix-{_fix_ctr[0]}",
                            ins=[],
                            outs=[],
                            engine=inst.engine,
                        )
                        nop.sync_info = bass_rust.SyncInfo(on_wait=[w], on_update=[])
                        out.append(nop)
                    inst.sync_info = bass_rust.SyncInfo(
                        on_wait=waits[-max_waits:], on_update=list(si.on_update)
                    )
                out.append(inst)
            blk.instructions = out


# ---------------------------------------------------------------------------
# Kernel program
# ---------------------------------------------------------------------------

def build_nc():
    nc = bass.Bass("TRN2", target_bir_lowering=False, debug=False)

    inT_d = nc.dram_tensor("inT", [128, 4, Bc], BF, kind="ExternalInput")
    h_d = nc.dram_tensor("h", [Bc, U], F32, kind="ExternalInput")
    hT_d = nc.dram_tensor("hT", [128, 8, Bc], BF, kind="ExternalInput")
    es_d = nc.dram_tensor("es", [8, 128, 4, ED], F8, kind="ExternalInput")
    esT_d = nc.dram_tensor("esT", [8, 128, 8, 512], F8, kind="ExternalInput")
    kernx_d = nc.dram_tensor("kernx", [XD, 3 * U], BF, kind="ExternalInput")
    kernc_d = nc.dram_tensor("kernc", [ED, 3 * U], F8, kind="ExternalInput")
    rk_d = nc.dram_tensor("rk", [U, 3 * U], BF, kind="ExternalInput")
    bias_d = nc.dram_tensor("bias", [3 * U], F32, kind="ExternalInput")
    wa_d = nc.dram_tensor("wab", [ED, U], F8, kind="ExternalInput")
    wat_d = nc.dram_tensor("wat", [8, 128, 8, 128], BF, kind="ExternalInput")
    va_d = nc.dram_tensor("va", [128, 2, 16], F8, kind="ExternalInput")
    out_d = nc.dram_tensor("out", [Bc, T + U], F32, kind="ExternalOutput")

    with tile.TileContext(nc) as tc:
        with (
            tc.tile_pool(name="singles", bufs=1) as sg,
            tc.tile_pool(name="esT", bufs=2) as esT_pool,
            tc.tile_pool(name="esn", bufs=3) as esn_pool,
            tc.tile_pool(name="tesT", bufs=2) as tesT_pool,
            tc.tile_pool(name="gT", bufs=1) as gT_pool,
            tc.tile_pool(name="gst", bufs=4) as gst_pool,
            tc.tile_pool(name="wk", bufs=2) as wk_pool,
            tc.tile_pool(name="kx", bufs=2) as kx_pool,
            tc.tile_pool(name="kc", bufs=3) as kc_pool,
            tc.tile_pool(name="smalls", bufs=4) as sm_pool,
            tc.tile_pool(name="ps_v", bufs=3, space="PSUM") as ps_v,
            tc.tile_pool(name="ps_tr", bufs=1, space="PSUM") as ps_tr,
            tc.tile_pool(name="ps_acc", bufs=2, space="PSUM") as ps_acc,
            tc.tile_pool(name="ps_e", bufs=1, space="PSUM") as ps_e,
            tc.tile_pool(name="ps_ct", bufs=1, space="PSUM") as ps_ct,
        ):
            # ---- startup DMAs, spread over engine queues: each dma_start
            # costs ~640ns of enqueue time on its issuing engine, so the
            # critical esT0/wab chunks get the sync queue to themselves ----
            hT_sb = sg.tile([128, 8, Bc], BF)
            nc.gpsimd.dma_start(out=hT_sb[:], in_=hT_d[:])

            # esT0 / wab chunk-interleaved: the first DR main matmul only
            # needs pair 0 of both.
            esT_cur = esT_pool.tile([128, 8, 512], F8, tag="esT", name="esT0")
            wab_sb = sg.tile([128, 8, U], F8)
            # first column-half of wab only: mains u0..u3 need wab cols
            # 0:512, so the critical startup prefix is 1MB, not 1.5MB
            for q in range(4):
                nc.sync.dma_start(
                    out=esT_cur[:, 2 * q:2 * q + 2, :],
                    in_=esT_d.ap()[0, :, 2 * q:2 * q + 2, :],
                )
                nc.sync.dma_start(
                    out=wab_sb[:, 2 * q:2 * q + 2, 0:512],
                    in_=wa_d.ap()[256 * q:256 * (q + 1), 0:512].rearrange(
                        "(i p) c -> p i c", i=2
                    ),
                )
            for q in range(4):
                nc.sync.dma_start(
                    out=wab_sb[:, 2 * q:2 * q + 2, 512:1024],
                    in_=wa_d.ap()[256 * q:256 * (q + 1), 512:1024].rearrange(
                        "(i p) c -> p i c", i=2
                    ),
                )

            va_sb = sg.tile([128, 2, 16], F8)
            nc.gpsimd.dma_start(out=va_sb[:], in_=va_d[:])
            # qk weight blocks: two enqueues on the sync queue AFTER the
            # critical esT0/wab chunks (priority = sync enqueue order)
            wau_all = sg.tile([128, 8, 8, 128], BF)
            for uh in range(2):
                nc.sync.dma_start(
                    out=wau_all[:, 4 * uh:4 * (uh + 1), :, :],
                    in_=wat_d.ap()[4 * uh:4 * (uh + 1)].rearrange(
                        "u p d c -> p u d c"
                    ),
                )
            h_sb = sg.tile([Bc, U], F32)
            nc.sync.dma_start(out=h_sb[:], in_=h_d[:])
            inT_sb = sg.tile([128, 4, Bc], BF)
            nc.sync.dma_start(out=inT_sb[:], in_=inT_d[:])

            def load_esT(g):
                t_ = esT_pool.tile([128, 8, 512], F8, tag="esT", name=f"esT{g}")
                nc.sync.dma_start(out=t_[:], in_=esT_d.ap()[g])
                return t_

            esn_tiles = {}

            def load_esn(g):
                t_ = esn_pool.tile([128, 4, ED], F8, tag="esn", name=f"esn{g}")
                nc.sync.dma_start(out=t_[:], in_=es_d.ap()[g])
                esn_tiles[g] = t_

            # warm the ACT table while the first DMAs are in flight
            scr1 = sg.tile([1, 1], F32)
            nc.vector.memset(scr1[:], 0.0)
            nc.scalar.activation(out=scr1[:], in_=scr1[:], func=AF.Tanh)

            bias_b = sg.tile([Bc, 3 * U], F32)
            # one partition-broadcast enqueue for the whole bias
            _bsrc = bias_d.ap()
            nc.gpsimd.dma_start(
                out=bias_b[:],
                in_=bass.AP(
                    tensor=_bsrc.tensor,
                    offset=_bsrc.offset,
                    ap=[[0, Bc], [1, 3 * U]],
                ),
            )

            esT_tiles = {0: esT_cur}

            thT = sg.tile([128, 8, Bc], BF)

            ident = sg.tile([16, 16], F32)
            make_identity(nc, ident[:])
            identb = sg.tile([Bc, Bc], BF)
            nc.vector.memset(identb[:], 0.0)
            nc.gpsimd.affine_select(
                out=identb[:],
                in_=identb[:],
                compare_op=mybir.AluOpType.not_equal,
                fill=1.0,
                base=0,
                pattern=[[-1, Bc]],
                channel_multiplier=1,
            )

            masks = sg.tile([128, 4, 8], F32)
            nc.vector.memset(masks[:], 0.0)
            for rr in range(4):
                nc.vector.memset(masks[0:64, rr, 2 * rr:2 * rr + 1], 1.0)
                nc.vector.memset(masks[64:128, rr, 2 * rr + 1:2 * rr + 2], 1.0)

            half_sb = sg.tile([Bc, 1], F32)
            nc.vector.memset(half_sb[:], 0.5)

            qkT = sg.tile([128, 8, Bc], BF)
            ctT = sg.tile([128, 8, Bc], F8)
            gh_sb = sg.tile([Bc, 2 * U], BF)
            gx0_sb = sg.tile([Bc, 3 * U], BF)
            ct_sb = sg.tile([Bc, ED], BF)       # rows 0..55 used (g0..g6)
            ct_sb7 = sg.tile([8, ED], BF)       # g7's rows (base-0 partitions)

            def qk_block_u(u):
                # qkT[u-block] = (Wa_top.T @ tanh(h).T) via small direct matmuls
                pq = ps_tr.tile([128, Bc], F32, tag="tr", name=f"pq{u}")
                for d in range(8):
                    nc.tensor.matmul(
                        pq[:],
                        wau_all[:, u, d, :],
                        thT[:, d, :],
                        start=(d == 0),
                        stop=(d == 7),
                    )
                nc.vector.tensor_copy(qkT[:, u, :], pq[:])

            # --- GRU weight-stream pieces: one batched enqueue per 512-col
            # slab, consumed d-granular by matmuls in the u-loop ---
            rkz_tiles = {}

            def load_rkz(n):
                # two half-slab enqueues so d=0 matmuls don't wait the full MB
                t_ = wk_pool.tile([128, 8, 512], BF, tag="wk", name=f"rkz{n}")
                for dh in range(2):
                    nc.sync.dma_start(
                        out=t_[:, 4 * dh:4 * (dh + 1), :],
                        in_=rk_d.ap()[
                            512 * dh:512 * (dh + 1), 512 * n:512 * (n + 1)
                        ].rearrange("(d p) c -> p d c", d=4),
                    )
                rkz_tiles[n] = t_

            def gh_step(n, d, pg):
                nc.tensor.matmul(
                    pg[:], hT_sb[:, d, :], rkz_tiles[n][:, d, :],
                    start=(d == 0), stop=(d == 7),
                )

            def gh_fin(n, pg):
                nc.vector.scalar_tensor_tensor(
                    out=gh_sb[:, 512 * n:512 * (n + 1)],
                    in0=pg[:],
                    scalar=1.0,
                    in1=bias_b[:, 512 * n:512 * (n + 1)],
                    op0=AluOpType.mult,
                    op1=AluOpType.add,
                )

            kx_tiles = {}

            def load_kx(n):
                t_ = kx_pool.tile([128, 4, 512], BF, tag="kx", name=f"kx{n}")
                for dh in range(2):
                    nc.sync.dma_start(
                        out=t_[:, 2 * dh:2 * (dh + 1), :],
                        in_=kernx_d.ap()[
                            256 * dh:256 * (dh + 1), 512 * n:512 * (n + 1)
                        ].rearrange("(d p) c -> p d c", d=2),
                    )
                kx_tiles[n] = t_

            def gx0_step(n, d, pa):
                nc.tensor.matmul(
                    pa[:], inT_sb[:, d, :], kx_tiles[n][:, d, :],
                    start=(d == 0), stop=(d == 3),
                )

            # --- pipelined e-acc -> alpha -> c_t pieces (lagged) ---
            pe_tiles = {}

            def eacc_pair(g, q, gT_g):
                # e[g] += (va_q).T @ gT[pair q]  -- fp8 DoubleRow, K=256
                if q == 0:
                    pe_tiles[g] = ps_e.tile([1, 512], F32, tag="e", name=f"pe{g}")
                nc.tensor.matmul(
                    pe_tiles[g][:],
                    va_sb[:, :, q:q + 1],
                    gT_g[:, 2 * q:2 * q + 2, :],
                    start=(q == 0),
                    stop=(q == 3),
                    perf_mode=DR,
                )

            def softmax(g):
                # alpha stays UNNORMALIZED; ct is scaled by CT_SCALE/sum via
                # srec8. exp undoes the VA_SCALE on e for free.
                e_sb = sm_pool.tile([1, 512], F32, tag="esb", name=f"esb{g}", bufs=1)
                nc.vector.tensor_copy(e_sb[:], pe_tiles[g][:])
                alpha = sm_pool.tile([8, T], F32, tag="al", name=f"al{g}")
                # scalar queue: hardware-dynamic and nearly empty, so this
                # latency-critical reshape never starves behind the bulk
                # weight slabs (gpsimd's software queue stalls ~7us at g6/g7)
                nc.scalar.dma_start(
                    out=alpha[:],
                    in_=e_sb[0:1, :].rearrange("p (b t) -> p b t", b=8),
                )
                ssum = sm_pool.tile([8, 1], F32, tag="ssum", name=f"ss{g}")
                nc.scalar.activation(
                    out=alpha[:], in_=alpha[:], func=AF.Exp,
                    scale=1.0 / VA_SCALE, accum_out=ssum[:],
                )
                srec = sm_pool.tile([8, 1], F32, tag="srec", name=f"sr{g}")
                nc.vector.reciprocal(srec[:], ssum[:])
                srec8 = sm_pool.tile([8, 1], F32, tag="srec8", name=f"sr8{g}")
                nc.vector.tensor_scalar_mul(srec8[:], srec[:], CT_SCALE)
                alpha_n = sm_pool.tile([8, T], F32, tag="aln", name=f"aln{g}")
                nc.vector.tensor_scalar_mul(alpha_n[:], alpha[:], srec[:])
                nc.gpsimd.dma_start(
                    out=out_d.ap()[8 * g:8 * (g + 1), 0:T], in_=alpha_n[:]
                )
                return alpha, srec8

            def ct_head(g, alpha):
                pat = ps_ct.tile([T, 8], F32, tag="ct", name=f"pat{g}")
                nc.tensor.transpose(pat[:], alpha[:], ident[:8, :8])
                alpT2 = sm_pool.tile([128, 8], F32, tag="alT2", name=f"aT2{g}")
                nc.vector.tensor_copy(alpT2[0:64, :], pat[:])
                nc.vector.tensor_copy(alpT2[64:128, :], pat[:])
                a2s = []
                for q in range(2):
                    a2 = sm_pool.tile([128, 2, 16], F8, tag="A", name=f"A{g}_{q}")
                    for i in range(2):
                        nc.gpsimd.tensor_mul(
                            a2[:, i, 0:8], alpT2[:], masks[:, 2 * q + i, :]
                        )
                    a2s.append(a2)
                return a2s

            def ct_tail(g, esn_g, a2s, srec8_g):
                # fp8 DR: out[8b, 512e] accumulating rr-pairs; g7's rows land
                # in the base-0 ct_sb7 tile directly, earlier rows go through
                # a staging tile + DMA (DVE writes need 32-aligned partition
                # bases).
                if g == 7:
                    stage = ct_sb7
                else:
                    stage = sm_pool.tile(
                        [8, ED], BF, tag="ctst", name=f"cts{g}", bufs=1
                    )
                for n in range(2):
                    pct = ps_ct.tile([8, 512], F32, tag="ct", name=f"pct{g}_{n}")
                    for q in range(2):
                        nc.tensor.matmul(
                            pct[:],
                            a2s[q][:, :, 0:8],
                            esn_g[:, 2 * q:2 * q + 2, 512 * n:512 * (n + 1)],
                            start=(q == 0),
                            stop=(q == 1),
                            perf_mode=DR,
                        )
                    nc.vector.tensor_scalar_mul(
                        stage[:, 512 * n:512 * (n + 1)], pct[:], srec8_g[:]
                    )
                if g != 7:
                    nc.gpsimd.dma_start(
                        out=ct_sb[8 * g:8 * (g + 1), :], in_=stage[:]
                    )

            # GRU ct-part weight preloads (fp8): one enqueue per gate part,
            # shaped [128, 4dcp, 2i, 1024] so DR pair slices come out 3D.
            kc_tiles = {}

            def preload_tail_part(part):
                wp = kc_pool.tile(
                    [128, 4, 2, U], F8, tag="kc", name=f"kc{part}"
                )
                nc.sync.dma_start(
                    out=wp[:],
                    in_=kernc_d.ap()[:, U * part:U * (part + 1)].rearrange(
                        "(q i p) c -> p q i c", q=4, i=2
                    ),
                )
                kc_tiles[part] = wp

            def emit_tesT_tanh(dst, src):
                for q in range(4):
                    nc.scalar.activation(
                        out=dst[:, 2 * q:2 * q + 2, :],
                        in_=src[:, 2 * q:2 * q + 2, :],
                        func=AF.Tanh,
                    )

            rkh_all = sg.tile([128, 8, U], BF)

            _dummy_ctr = [0]

            def dummy_mm(k):
                # dep-free 64-col transposes: ~150ns each of PE activity to
                # keep the HAM clock warm across short serial waits
                for _ in range(k):
                    _dummy_ctr[0] += 1
                    pdm = ps_tr.tile(
                        [Bc, Bc], BF, tag="tr", name=f"dum{_dummy_ctr[0]}"
                    )
                    nc.tensor.transpose(pdm[:], identb[:], identb[:])

            # ---- attention superblock loop ----
            load_rkz(0)
            tesT_cur = tesT_pool.tile([128, 8, 512], F8, tag="tesT", name="tes0")
            emit_tesT_tanh(tesT_cur, esT_cur)
            nc.scalar.activation(out=thT[:], in_=hT_sb[:], func=AF.Tanh)

            gT_tiles = {}
            alpha_info = {}   # g -> (alpha, srec8)
            a2s_prev = None

            for g in range(NSBLK):
                gh_n = g - 1 if 1 <= g <= 4 else None
                # hh-column chunks (n=4,5) move to the post-loop tail as
                # g7-independent PE filler
                gx_n = g - 2 if 2 <= g <= 5 else None
                pg = (
                    ps_acc.tile([Bc, 512], F32, tag="acc", name=f"pg{gh_n}")
                    if gh_n is not None else None
                )
                pa = (
                    ps_acc.tile([Bc, 512], F32, tag="acc", name=f"gx0_{gx_n}")
                    if gx_n is not None else None
                )

                tesT = tesT_cur
                ilv = 1 <= g < NSBLK - 1   # interleave next tesT tanh (g>=1)
                if ilv:
                    tesT_nxt = tesT_pool.tile(
                        [128, 8, 512], F8, tag="tesT", name=f"tes{g+1}"
                    )
                gT = gT_pool.tile([128, 8, 512], F8, tag="gT", name=f"gt{g}")
                gT_tiles[g] = gT
                gstage = gst_pool.tile(
                    [128, 2, 512], BF, tag="gst", name=f"gs{g}"
                )
                for u in range(8):
                    pv = ps_v.tile([128, 512], F32, tag="v", name=f"pv{g}_{u}")
                    for j2 in range(4):
                        # fp8 DoubleRow: contracts 256 rows per pass
                        nc.tensor.matmul(
                            pv[:],
                            wab_sb[:, 2 * j2:2 * j2 + 2, 128 * u:128 * (u + 1)],
                            tesT[:, 2 * j2:2 * j2 + 2, :],
                            start=(j2 == 0),
                            stop=(j2 == 3),
                            perf_mode=DR,
                        )
                    if g == 0:
                        # must precede the STT below, which reads qkT[:, u, :]
                        qk_block_u(u)
                    qk_slice = qkT[:, u, 8 * g:8 * g + 8]
                    qk_bc = bass.AP(
                        tensor=qk_slice.tensor,
                        offset=qk_slice.offset,
                        ap=[
                            list(qk_slice.ap[0]),
                            list(qk_slice.ap[1]),
                            [0, T],
                        ],
                    )
                    nc.vector.scalar_tensor_tensor(
                        out=gstage[:, u % 2, :],
                        in0=pv[:],
                        scalar=1.0 / WA_SCALE,
                        in1=qk_bc,
                        op0=AluOpType.mult,
                        op1=AluOpType.add,
                    )
                    if u % 2 == 1:
                        # batched pair tanh -> fp8 gT
                        nc.scalar.activation(
                            out=gT[:, u - 1:u + 1, :],
                            in_=gstage[:],
                            func=AF.Tanh,
                        )
                        gstage = gst_pool.tile(
                            [128, 2, 512], BF, tag="gst", name=f"gs{g}_{u}"
                        )
                        if ilv:
                            # next superblock's tesT tanh pair
                            qq = (u - 1) // 2
                            nc.scalar.activation(
                                out=tesT_nxt[:, 2 * qq:2 * qq + 2, :],
                                in_=esT_tiles[g + 1][:, 2 * qq:2 * qq + 2, :],
                                func=AF.Tanh,
                            )
                    # lagged e-acc pairs: q0/q1 of g at u=5/7, q2/q3 of g-1
                    # at u=0/2. softmax starts at u=2 and ct_head waits to
                    # u=6 so the alpha-reshape DMA's software-queue latency
                    # jitter (multi-us) never stalls the PE at pat.
                    if u == 5:
                        eacc_pair(g, 0, gT)
                    elif u == 7:
                        eacc_pair(g, 1, gT)
                    elif u == 0 and g >= 1:
                        eacc_pair(g - 1, 2, gT_tiles[g - 1])
                    elif u == 2 and g >= 1:
                        eacc_pair(g - 1, 3, gT_tiles[g - 1])
                    if u == 2 and g >= 1:
                        alpha_info[g - 1] = softmax(g - 1)
                    if u == 5 and g >= 1:
                        a2s_prev = ct_head(g - 1, alpha_info[g - 1][0])
                    if u == 7 and g >= 1:
                        ct_tail(
                            g - 1, esn_tiles[g - 1], a2s_prev,
                            alpha_info[g - 1][1],
                        )
                    if gh_n is not None:
                        gh_step(gh_n, u, pg)
                    if gx_n is not None and u < 4:
                        gx0_step(gx_n, u, pa)

                if gh_n is not None:
                    gh_fin(gh_n, pg)
                if gx_n is not None:
                    # fold gh (z/r cols) into the gx0 store so the GRU tail
                    # does a single add per gate column
                    nc.vector.scalar_tensor_tensor(
                        out=gx0_sb[:, 512 * gx_n:512 * (gx_n + 1)],
                        in0=pa[:],
                        scalar=1.0,
                        in1=gh_sb[:, 512 * gx_n:512 * (gx_n + 1)],
                        op0=AluOpType.mult,
                        op1=AluOpType.add,
                    )

                if ilv:
                    tesT_cur = tesT_nxt
                elif g == 0:
                    # g0: batch-tanh tesT(1) after the u-loop (esT1 lands late)
                    esT_tiles[1] = load_esT(1)
                    tesT_cur = tesT_pool.tile(
                        [128, 8, 512], F8, tag="tesT", name="tes1"
                    )
                    emit_tesT_tanh(tesT_cur, esT_tiles[1])
                if g + 2 < NSBLK:
                    esT_tiles[g + 2] = load_esT(g + 2)

                # spread remaining weight streams across the attention phase
                if g == 0:
                    load_esn(0)
                    load_esn(1)
                    load_esn(2)
                elif g + 2 < NSBLK:
                    load_esn(g + 2)
                if 1 <= g <= 3:
                    load_rkz(g)      # h@rk slab for gh at g+1
                if 1 <= g <= 6:
                    load_kx(g - 1)   # inputs@kernel slab for gx0 at g+1
                if g == 5:
                    preload_tail_part(0)
                if g == 6:
                    preload_tail_part(1)
                    nc.sync.dma_start(
                        out=rkh_all[:],
                        in_=rk_d.ap()[:, 2 * U:].rearrange(
                            "(d p) c -> p d c", d=8
                        ),
                    )
                if g == 7:
                    preload_tail_part(2)

            # ---- post-loop: finish g7's e/alpha/ct; keep the PE dense with
            # g7-independent filler so the HAM clock never drops ----
            eacc_pair(7, 2, gT_tiles[7])
            eacc_pair(7, 3, gT_tiles[7])
            alpha_info[7] = softmax(7)
            dummy_mm(4)

            # real filler: inputs-part gx0 for the hh columns (n=4,5)
            for n45 in (4, 5):
                pa45 = ps_acc.tile([Bc, 512], F32, tag="acc", name=f"gx0_{n45}")
                for d in range(4):
                    gx0_step(n45, d, pa45)
                nc.vector.scalar_tensor_tensor(
                    out=gx0_sb[:, 512 * n45:512 * (n45 + 1)],
                    in0=pa45[:],
                    scalar=1.0,
                    in1=bias_b[:, 512 * n45:512 * (n45 + 1)],
                    op0=AluOpType.mult,
                    op1=AluOpType.add,
                )

            # pre-transpose ct rows 0..55 (g0..g6) into fp8 ctT -- fills the
            # PE while g7's softmax chain runs on gpsimd/ACT/DVE
            for j in range(8):
                pool, tag = (ps_tr, "tr") if j % 2 == 0 else (ps_ct, "ct")
                pt = pool.tile([128, 56], BF, tag=tag, name=f"tpc_{j}")
                nc.tensor.transpose(
                    pt[:], ct_sb[0:56, 128 * j:128 * (j + 1)],
                    identb[0:56, 0:56],
                )
                nc.scalar.copy(ctT[:, j, 0:56], pt[:])
            dummy_mm(4)
            a2s_prev = ct_head(7, alpha_info[7][0])
            dummy_mm(3)
            ct_tail(7, esn_tiles[7], a2s_prev, alpha_info[7][1])
            dummy_mm(4)

            # ---- GRU tail ----
            z_sb = sg.tile([Bc, U], F32)
            r_sb = sg.tile([Bc, U], F32)
            rh_bf = sg.tile([Bc, U], BF)
            rhT = sg.tile([128, 8, Bc], BF)
            zh_sb = sg.tile([Bc, U], F32)
            omz_sb = sg.tile([Bc, U], F32)
            hh_sb = r_sb  # r is dead once rh_bf is formed

            # six gate accumulators in the (now idle) attention psum banks
            gx = [
                ps_v.tile([Bc, 512], F32, tag="v", name="gxa"),
                ps_v.tile([Bc, 512], F32, tag="v", name="gxb"),
                ps_v.tile([Bc, 512], F32, tag="v", name="gxc"),
                ps_e.tile([Bc, 512], F32, tag="e", name="gxd"),
                ps_acc.tile([Bc, 512], F32, tag="acc", name="gxe"),
                ps_acc.tile([Bc, 512], F32, tag="acc", name="gxf"),
            ]

            # last 8 ct rows: transpose + copy-cast
            for j in range(8):
                pool, tag = (ps_tr, "tr") if j % 2 == 0 else (ps_ct, "ct")
                pt = pool.tile([128, 8], BF, tag=tag, name=f"tp8_{j}")
                nc.tensor.transpose(
                    pt[:], ct_sb7[:, 128 * j:128 * (j + 1)], identb[0:8, 0:8]
                )
                nc.scalar.copy(ctT[:, j, 56:64], pt[:])
            dummy_mm(3)

            # z/r ct-part: fp8 DR pairs (scale 1/(KC*CT) folded into the
            # adds). r gates (part 1) stream FIRST: their psums gate the
            # rhT transposes, so finishing them early overlaps the r-gate
            # chain with the z and hh-ct streams.
            for part in (1, 0):
                for dcp in range(4):
                    for half in range(2):
                        n = 2 * part + half
                        nc.tensor.matmul(
                            gx[n][:],
                            ctT[:, 2 * dcp:2 * dcp + 2, :],
                            kc_tiles[part][:, dcp, :, 512 * half:512 * (half + 1)],
                            start=(dcp == 0),
                            stop=(dcp == 3),
                            perf_mode=DR,
                        )

            GATE_RS = 1.0 / (KC_SCALE * CT_SCALE)

            def add_inplace(pa_, n, src_sb, scalar):
                nc.vector.scalar_tensor_tensor(
                    out=pa_[:],
                    in0=pa_[:],
                    scalar=scalar,
                    in1=src_sb[:, 512 * n:512 * (n + 1)],
                    op0=AluOpType.mult,
                    op1=AluOpType.add,
                )

            # hh ct-part stream first (keeps PE busy while the z/r gate chain
            # runs on ACT/DVE); rh contributions are pre-scaled x512 so the
            # shared psum stays on one scale
            for dcp in range(4):
                for half in range(2):
                    nc.tensor.matmul(
                        gx[4 + half][:],
                        ctT[:, 2 * dcp:2 * dcp + 2, :],
                        kc_tiles[2][:, dcp, :, 512 * half:512 * (half + 1)],
                        start=(dcp == 0),
                        stop=False,
                        perf_mode=DR,
                    )

            # gates: hard_sigmoid(gx + gx0 + gh) = min(relu(0.2x+0.5), 1).
            # r gates FIRST -- only r gates the PE-critical rhT transposes;
            # z feeds nothing until the final combine.
            dummy_mm(6)
            for n in (2, 3, 0, 1):
                dst = z_sb if n < 2 else r_sb
                o = 512 * (n % 2)
                sl = slice(o, o + 512)
                if n == 2:
                    # r-half0 gates the FIRST rhT transposes: run its chain
                    # in 256-col strips so ACT/DVE pipeline and transpose j0
                    # starts ~1.3us earlier (keeps the HAM clock warm)
                    for s2 in range(2):
                        ps = slice(256 * s2, 256 * (s2 + 1))
                        ss = slice(o + 256 * s2, o + 256 * (s2 + 1))
                        nc.vector.scalar_tensor_tensor(
                            out=gx[n][:, ps], in0=gx[n][:, ps],
                            scalar=GATE_RS,
                            in1=gx0_sb[:, 512 * n + 256 * s2:512 * n + 256 * (s2 + 1)],
                            op0=AluOpType.mult, op1=AluOpType.add,
                        )
                        nc.scalar.activation(
                            out=dst[:, ss], in_=gx[n][:, ps],
                            func=AF.Relu, bias=half_sb[:], scale=0.2,
                        )
                        nc.vector.tensor_scalar_min(dst[:, ss], dst[:, ss], 1.0)
                        nc.vector.scalar_tensor_tensor(
                            out=rh_bf[:, ss], in0=dst[:, ss],
                            scalar=1.0 / GATE_RS, in1=h_sb[:, ss],
                            op0=AluOpType.mult, op1=AluOpType.mult,
                        )
                        for j in range(2 * s2, 2 * s2 + 2):
                            pool, tag = (ps_tr, "tr") if j % 2 == 0 else (ps_ct, "ct")
                            pt = pool.tile([128, Bc], BF, tag=tag, name=f"tprh_{j}")
                            nc.tensor.transpose(
                                pt[:], rh_bf[:, 128 * j:128 * (j + 1)], identb[:]
                            )
                            nc.scalar.copy(rhT[:, j, :], pt[:])
                    continue
                add_inplace(gx[n], n, gx0_sb, GATE_RS)
                nc.scalar.activation(
                    out=dst[:, sl], in_=gx[n][:],
                    func=AF.Relu, bias=half_sb[:], scale=0.2,
                )
                nc.vector.tensor_scalar_min(dst[:, sl], dst[:, sl], 1.0)
                if n < 2:
                    # precompute z*h and (1-z) off the critical path
                    nc.vector.tensor_mul(zh_sb[:, sl], dst[:, sl], h_sb[:, sl])
                    nc.vector.tensor_scalar(
                        out=omz_sb[:, sl], in0=dst[:, sl],
                        scalar1=-1.0, scalar2=1.0,
                        op0=AluOpType.mult, op1=AluOpType.add,
                    )
                else:
                    # rh = (r * 512) * h so the hh psum (ct-part x512) stays
                    # single-scale
                    nc.vector.scalar_tensor_tensor(
                        out=rh_bf[:, sl],
                        in0=dst[:, sl],
                        scalar=1.0 / GATE_RS,
                        in1=h_sb[:, sl],
                        op0=AluOpType.mult,
                        op1=AluOpType.mult,
                    )
                    for j in range(4 * (n - 2), 4 * (n - 1)):
                        pool, tag = (ps_tr, "tr") if j % 2 == 0 else (ps_ct, "ct")
                        pt = pool.tile([128, Bc], BF, tag=tag, name=f"tprh_{j}")
                        nc.tensor.transpose(
                            pt[:], rh_bf[:, 128 * j:128 * (j + 1)], identb[:]
                        )
                        nc.scalar.copy(rhT[:, j, :], pt[:])

            # (r*h) @ rk_hh stream, bank-major so the first hh half finishes
            # early and its scalar/vector chain overlaps the second bank
            for n2 in range(2):
                for d in range(8):
                    nc.tensor.matmul(
                        gx[4 + n2][:],
                        rhT[:, d, :],
                        rkh_all[:, d, 512 * n2:512 * (n2 + 1)],
                        start=False,
                        stop=(d == 7),
                    )

            # hh = tanh(...); h_new = z*h + (1-z)*hh  (zh/omz precomputed)
            t1 = sg.tile([Bc, U], F32)
            for n2 in range(2):
                o = 512 * n2
                sl = slice(o, o + 512)
                pa = gx[4 + n2]
                add_inplace(pa, 4 + n2, gx0_sb, GATE_RS)
                nc.scalar.activation(out=hh_sb[:, sl], in_=pa[:], func=AF.Tanh)
                nc.vector.tensor_mul(t1[:, sl], omz_sb[:, sl], hh_sb[:, sl])
                nc.vector.tensor_add(t1[:, sl], t1[:, sl], zh_sb[:, sl])
                nc.sync.dma_start(out=out_d.ap()[:, T + o:T + o + 512], in_=t1[:, sl])

    return nc


_built = [None]


def _get_nc():
    if _built[0] is None:
        nc = build_nc()
        fix_multi_waits(nc)
        _built[0] = nc
    return _built[0]


def make_in_maps(inputs):
    import ml_dtypes

    bf16 = ml_dtypes.bfloat16
    f8 = ml_dtypes.float8_e4m3

    def f32(name):
        return np.ascontiguousarray(np.asarray(inputs[name], dtype=np.float32))

    inp = f32("inputs")
    h = f32("h")
    es = f32("encoder_states")
    ker = f32("kernel")
    rk = f32("recurrent_kernel").astype(bf16)
    bias = f32("bias")
    wa = f32("Wa")
    va = f32("Va")

    kernx = np.ascontiguousarray(ker[:XD]).astype(bf16)
    kernc = np.ascontiguousarray(ker[XD:] * KC_SCALE).astype(f8)

    # va2[p, i, q] = va[128*(2q+i)+p] * VA_SCALE  (cols q>=4 unused)
    va2 = np.zeros((128, 2, 16), np.float32)
    va2[:, :, 0:4] = (va[:, 0] * VA_SCALE).reshape(4, 2, 128).transpose(2, 1, 0)
    va2 = va2.astype(f8)

    wab = np.ascontiguousarray(wa[U:] * WA_SCALE).astype(f8)  # [1024, 1024]
    # wat [8u, 128p, 8j, 128c]: wat[u, p, j, :] = wa_top[128j+p, 128u:128u+128]
    wat = np.ascontiguousarray(
        wa[:U].astype(bf16).reshape(8, 128, 8, 128).transpose(2, 1, 0, 3)
    )

    in_maps = []
    for c in range(N_CORES):
        sl = slice(c * Bc, (c + 1) * Bc)
        es_c = es[sl].reshape(Bc * T, ED).astype(bf16)
        h_c = h[sl]
        # preshaped SBUF layouts: X.T [D, Bc] -> [128, D//128, Bc] (p, j, b)
        inT_c = inp[sl].T.reshape(4, 128, Bc).transpose(1, 0, 2)
        hT_c = h_c.T.reshape(8, 128, Bc).transpose(1, 0, 2)
        # es_pre [8g, 128p, 4r, 1024e]: es_pre[g, p, r] = es_c[512g+128r+p]
        es_pre = es_c.reshape(8, 4, 128, ED).transpose(0, 2, 1, 3).astype(f8)
        # esT_pre [8g, 128p, 8j, 512t]: esT_pre[g, p, j, t] = es_c[512g+t, 128j+p]
        esT_pre = np.ascontiguousarray(es_c.T).reshape(8, 128, 8, 512)
        esT_pre = esT_pre.transpose(2, 1, 0, 3).astype(f8)
        in_maps.append({
            "inT": np.ascontiguousarray(inT_c).astype(bf16),
            "h": h_c,
            "hT": np.ascontiguousarray(hT_c).astype(bf16),
            "es": np.ascontiguousarray(es_pre),
            "esT": np.ascontiguousarray(esT_pre),
            "kernx": kernx,
            "kernc": kernc,
            "rk": rk,
            "bias": bias,
            "wab": wab,
            "wat": wat,
            "va": va2,
        })
    return in_maps


def kernel(**inputs):
    nc = _get_nc()

    from concourse.bass_utils import run_bass_kernel_spmd

    in_maps = make_in_maps(inputs)
    res = run_bass_kernel_spmd(nc, in_maps, list(range(N_CORES)))
    out = np.concatenate(
        [res.results[c]["out"] for c in range(N_CORES)], axis=0
    ).astype(np.float32)
    return out
